# revision 14
# baseline (speedup 1.0000x reference)
"""ContextAwareSpanClassifier Trainium2 Bass kernel (bf16, software-pipelined).

Problem (hardcoded): B=4, S=2048, H=768, L=9, M=5 (window W=11).
  base_logits = x @ Wc + bc
  s = x . wa + ba ; windowed softmax over [t-5, t+5] (seq-edge masked)
  ctx[t] = sum_o attn[t,o] * x[t+o]
  h = gelu_erf(LN(cat(x,ctx) @ W1 + b1) * gamma + beta)
  out = 0.5*base_logits + 0.5*(h @ W2 + b2)

Sharding: data parallel over B*S = 8192 tokens -> 8 cores x 1024 tokens
(core c: batch c//2, seq half (c%2)*1024) with 5-token zero-padded halos.
Params replicated. ba shift cancels in softmax and is otherwise unused.

Three-deep software pipeline: each unrolled step emits stages for three
bodies so the PE queue never waits on a cross-engine chain:
  step(i): Aload_i   loads for body i (xT XBAR transpose, params, x_sb)
           B_{i-1}   softmax denominator banded (PE) -> recip -> A-muls
           C1_{i-2}  LN stats matmuls (PE; covers B's recip/A DVE latency)
           D1_{i-1}  ctx banded (PE) + ctxT copies (DVE/ACT/Pool rotate)
           Acomp_i   scores sweep (PE; xT DMA has had a step of lead),
                     pt_sc transposes, softmax exp -> e_col -> U tiles
           C2_{i-2}  LN scalars (ln+exp rstd: shares the ACT exp table
                     with softmax -> only 2 table loads/step), apply, gelu
           D2_{i-1}  W1 both 512-chunks interleaved (each stationary
                     loaded once), h copies + hsq
           E_{i-2}   W2 proj, base+ctx combine, out transposes, store
PE stream/step: denom, stats, ctx, scores, W1, W2 -- every cross-engine
wait is covered by PE work from a neighboring body. 2 ACT table loads
per step (exp-set: softmax exp + LN ln/exp-rstd; gelu-set: gelu, with
Identity/Copy ops valid in any set). B/C/E read params from the newest
loaded generation (values identical) so pb/pf need only 2 buffers.
"""

from contextlib import ExitStack

import numpy as np
import ml_dtypes

import concourse.bass as bass
import concourse.tile as tile
from concourse import bacc, mybir
from concourse.bass_utils import run_bass_kernel_spmd

F32 = mybir.dt.float32
BF16 = mybir.dt.bfloat16
AF = mybir.ActivationFunctionType
ALU = mybir.AluOpType

B, S, H = 4, 2048, 768
L, M = 9, 5
TOK = 1024             # tokens per core
NT = 8                 # 128-token output tiles per core
NJ = 9                 # x storage tiles (tile 8 has 10 valid rows)
FLAT = TOK + 2 * M     # 1034
FPAD = 1040
HC = H // 128          # 6
KC = 2 * H // 128      # 12
WB = 2 * M + 1         # 11
UW = 138               # skew-band width: 128 + 10
EPS = 1e-5
UNROLL = 32

# bf16 param blob column offsets (small constants first so the leading DMA
# slice unblocks transposes/scores while the big W1 slices stream in)
WAC = 0                       # [128, HC*33] per-k [Wc | pad | wa@32] stationaries
ONEC = WAC + HC * 33          # [128, 128] ones
MSKC = ONEC + 128             # [128, UW] skew-band mask
W1C = MSKC + UW               # [128, KC*H] w1[p, k*H + m] = W1[k*128+p, m]
WSTC = W1C + KC * H           # [128, HC*L] W2 k-slabs (gl part only)
PB2 = WSTC + HC * L
SMALL = W1C                   # leading small-constant slice width

# f32 param blob column offsets
EMC = 0                       # [128, NJ] edge mask (per-core)
B1C = EMC + NJ                # [128, HC]
GAC = B1C + HC                # [128, HC]
BEC = GAC + HC                # [128, HC]
B9C = BEC + HC                # [:9] bias9 = bc + b2
E0C = B9C + 1                 # [128,1] e0 basis column
ID9C = E0C + 1                # [:9, 9] eye(9)
EPSC = ID9C + L               # [128,1] eps
PF = EPSC + 1

INV_H = 1.0 / H


def make_pools(tc, ctx):
    p = {}
    p["const"] = ctx.enter_context(tc.tile_pool(name="const", bufs=2))
    p["persist"] = ctx.enter_context(tc.tile_pool(name="persist", bufs=2))
    p["one"] = ctx.enter_context(tc.tile_pool(name="one", bufs=1))
    p["small"] = ctx.enter_context(tc.tile_pool(name="small", bufs=2))
    p["ln"] = ctx.enter_context(tc.tile_pool(name="ln", bufs=4))
    p["lt"] = ctx.enter_context(tc.tile_pool(name="lt", bufs=3))
    p["ps_mm"] = ctx.enter_context(tc.tile_pool(name="ps_mm", bufs=3, space="PSUM"))
    p["ps_sm"] = ctx.enter_context(tc.tile_pool(name="ps_sm", bufs=3, space="PSUM"))
    p["ps_st"] = ctx.enter_context(tc.tile_pool(name="ps_st", bufs=2, space="PSUM"))
    return p


def pviews(st):
    pb, pf = st["pb"], st["pf"]
    return {
        "w1_v": pb[:, W1C:W1C + KC * H].rearrange("p (k m) -> p k m", k=KC),
        "wst_v": pb[:, WSTC:WSTC + HC * L].rearrange("p (k l) -> p k l", k=HC),
        "wa_v": pb[:, WAC:WAC + HC * 33].rearrange("p (k c) -> p k c", k=HC),
        "ones_v": pb[:, ONEC:ONEC + 128],
        "msk_v": pb[:, MSKC:MSKC + UW],
        "emask": pf[:, EMC:EMC + NJ],
        "b1_v": pf[:, B1C:B1C + HC],
        "ga_v": pf[:, GAC:GAC + HC],
        "be_v": pf[:, BEC:BEC + HC],
        "bias9": pf[:L, B9C:B9C + 1],
        "e0": pf[:, E0C:E0C + 1],
        "id9": pf[:L, ID9C:ID9C + L],
        "eps_v": pf[:, EPSC:EPSC + 1],
    }


def stage_Aload(nc, p, io):
    """Issue all DMA loads for a new body; returns its state dict.

    xT arrives pre-transposed from the host (plain DMA, no XBAR); big
    transfers are spread across SP/ACT/DVE HWDGE queues so multiple DMA
    rings run in parallel."""
    (xt_d, x_d, pb_d, pf_d, out_d) = io
    ppool, cpool = p["persist"], p["const"]
    st = {}
    st["xT"] = ppool.tile([128, HC, NJ * 128], BF16, tag="xT", name="xT")
    xt_view = st["xT"].rearrange("p h f -> p (h f)")
    half = HC * NJ * 128 // 2
    nc.sync.dma_start(out=xt_view[:, :half], in_=xt_d[:, :half])
    nc.sync.dma_start(out=xt_view[:, half:], in_=xt_d[:, half:])
    st["pb"] = cpool.tile([128, PB2], BF16, tag="pb", name="pb")
    nc.scalar.dma_start(out=st["pb"][:, :SMALL], in_=pb_d[:, :SMALL])
    st["pf"] = cpool.tile([128, PF], F32, tag="pf", name="pf")
    nc.scalar.dma_start(out=st["pf"], in_=pf_d)
    st["x_sb"] = ppool.tile([128, NJ, H], BF16, tag="x", name="x_sb")
    x_view = x_d.rearrange("(j p) h -> p j h", p=128)
    for a, b in ((0, 5), (5, 9)):
        nc.gpsimd.dma_start(out=st["x_sb"][:, a:b, :], in_=x_view[:, a:b, :])
    for (a, b), eng in (((W1C, W1C + 4 * H), nc.scalar),
                        ((W1C + 4 * H, W1C + 8 * H), nc.gpsimd),
                        ((W1C + 8 * H, PB2), nc.sync)):
        eng.dma_start(out=st["pb"][:, a:b], in_=pb_d[:, a:b])
    st["out_view"] = out_d.rearrange("(j p) l -> p j l", p=128)
    return st


def stage_Acomp(nc, p, st):
    """Scores/base sweep + pt_sc transposes + softmax exp/e_col/U tiles."""
    v = pviews(st)
    ppool, spool = p["persist"], p["small"]
    ps_mm, ps_sm = p["ps_mm"], p["ps_sm"]

    sb10 = ppool.tile([33, FPAD], F32, tag="sb10", name="sb10", bufs=3)
    st["sb10"] = sb10
    pt_sc = ps_sm.tile([128, 512], F32, tag="sm", name="pt_sc")
    nc.vector.memset(pt_sc[:, :16], 0.0)

    for ci in range(3):
        c0, n = ((0, 512), (512, 512), (1024, 10))[ci]
        ps = ps_mm.tile([128, 512], F32, tag="mm", name="sc_ps")
        for hc in range(HC):
            nc.tensor.matmul(ps[:33, :n], v["wa_v"][:, hc, :],
                             st["xT"][:, hc, c0:c0 + n],
                             start=(hc == 0), stop=(hc == HC - 1))
        if ci % 2:
            nc.vector.tensor_copy(out=sb10[:, c0:c0 + n], in_=ps[:33, :n])
        else:
            nc.scalar.copy(out=sb10[:, c0:c0 + n], in_=ps[:33, :n])
    for j in range(NJ):
        n = 128 if j < NJ - 1 else 10
        nc.tensor.transpose(pt_sc[:n, j:j + 1],
                            sb10[32:33, 128 * j:128 * j + n],
                            v["e0"][32:33, :])

    e_tmp = spool.tile([128, NJ], F32, tag="etmp", name="e_tmp")
    nc.scalar.activation(out=e_tmp, in_=pt_sc[:, :NJ], func=AF.Exp)
    e_col = spool.tile([128, NJ], F32, tag="ecol", name="e_col")
    nc.vector.tensor_mul(out=e_col, in0=e_tmp, in1=v["emask"])
    U = [None] * NJ
    for j in range(NJ):
        u = spool.tile([128, UW], BF16, tag=f"u{j}", name=f"u{j}", bufs=2)
        nc.gpsimd.tensor_scalar_mul(out=u, in0=v["msk_v"],
                                    scalar1=e_col[:, j:j + 1])
        U[j] = u
    st["U"] = U


def banded(nc, out_bank, half, lhs_of_j, rhs_of_j):
    # out_bank covers dst cols [512*half, 512*half+512); src tile j
    # contributes dst cols [128j-10, 128j+128), 10-col overlaps accumulate
    # in PSUM (start only on the bank's first writer)
    base = 512 * half
    first = True
    for j in range(4 * half, 4 * half + 5):
        lo = max(128 * j - 10, 0)
        hi = min(128 * j + 128, TOK)
        lo, hi = max(lo, base), min(hi, base + 512)
        if lo >= hi:
            continue
        ua = lo - (128 * j - 10)
        nc.tensor.matmul(out_bank[:, lo - base:hi - base],
                         lhs_of_j(j), rhs_of_j(j)[:, ua:ua + hi - lo],
                         start=first, stop=(j == 4 * half + 4))
        first = False


def stage_B(nc, p, st, pst):
    """Softmax denominator banded (PE) -> reciprocal -> A-muls (in U)."""
    v = pviews(pst)
    ppool, ps_sm = p["persist"], p["ps_sm"]
    U = st["U"]
    r_rep = ppool.tile([128, TOK], BF16, tag="rrep", name="r_rep")
    st["r_rep"] = r_rep
    pds = []
    for half in range(2):
        pd = ps_sm.tile([128, 512], F32, tag="sm", name="pd")
        banded(nc, pd, half, lambda j: v["ones_v"], lambda j: U[j])
        pds.append(pd)
    with nc.allow_low_precision(reason="bf16 softmax denom recip"):
        for half in range(2):
            nc.vector.reciprocal(out=r_rep[:, 512 * half:512 * half + 512],
                                 in_=pds[half])
    # A_j = U_j * r built in place (disjoint column pieces per half)
    for half in range(2):
        base = 512 * half
        for j in range(4 * half, 4 * half + 5):
            glo = max(128 * j - 10, base)
            ghi = min(128 * j + 128, base + 512)
            if glo >= ghi:
                continue
            ua = glo - (128 * j - 10)
            nc.vector.tensor_mul(out=U[j][:, ua:ua + ghi - glo],
                                 in0=U[j][:, ua:ua + ghi - glo],
                                 in1=r_rep[:, glo:ghi])


def stage_C1(nc, p, st, pst):
    """LN stats matmuls on PE (sum h, sum h^2 per 512-chunk)."""
    v = pviews(pst)
    ps_st = p["ps_st"]
    hs, qs = st["hs"], st["qs"]
    st["ps_stats"] = []
    for cch in range(2):
        ps_s = ps_st.tile([128, 512], F32, tag="st", name="ps_s")
        for m in range(HC):
            nc.tensor.matmul(ps_s, v["ones_v"], hs[cch][:, m, :],
                             start=(m == 0), stop=(m == HC - 1))
        ps_q = ps_st.tile([128, 512], F32, tag="st", name="ps_q")
        for m in range(HC):
            nc.tensor.matmul(ps_q, v["ones_v"], qs[cch][:, m, :],
                             start=(m == 0), stop=(m == HC - 1))
        st["ps_stats"].append((ps_s, ps_q))


def stage_C2a(nc, p, st, pst):
    """Drain the four stats PSUM banks immediately (mu via ACT, var via
    DVE) so the bank rotation never waits on later-queued engine work."""
    lnpool = p["ln"]
    st["lnmv"] = []
    for cch in range(2):
        ps_s, ps_q = st["ps_stats"][cch]
        mu = lnpool.tile([128, 512], F32, tag="lnmu", name="mu", bufs=2)
        nc.scalar.activation(out=mu, in_=ps_s, func=AF.Copy, scale=INV_H)
        musq = lnpool.tile([128, 512], F32, tag="ln", name="musq")
        nc.vector.tensor_mul(out=musq, in0=mu, in1=mu)
        var = lnpool.tile([128, 512], F32, tag="ln", name="var")
        nc.vector.scalar_tensor_tensor(out=var, in0=ps_q, scalar=INV_H,
                                       in1=musq, op0=ALU.mult,
                                       op1=ALU.subtract)
        st["lnmv"].append((mu, var))


def stage_C2(nc, p, st, pst):
    """LN scalars (sqrt+recip rstd), apply affine, gelu."""
    v = pviews(pst)
    lnpool, ltpool, gpool = p["ln"], p["lt"], p["one"]
    hs = st["hs"]
    gls = []
    lns = []
    for cch in range(2):
        mu, var = st["lnmv"][cch]
        sd = lnpool.tile([128, 512], F32, tag="ln", name="sd")
        nc.scalar.activation(out=sd, in_=var, func=AF.Sqrt, bias=v["eps_v"])
        rstd = lnpool.tile([128, 512], BF16, tag="lnb", name="rstd")
        with nc.allow_low_precision(reason="bf16 rstd is within LN tolerance"):
            nc.vector.reciprocal(out=rstd, in_=sd)
        bln = lnpool.tile([128, 512], BF16, tag="lnb", name="bln")
        nc.vector.scalar_tensor_tensor(out=bln, in0=mu, scalar=-1.0,
                                       in1=rstd, op0=ALU.mult, op1=ALU.mult)
        lns.append((rstd, bln))
    for cch in range(2):
        rstd, bln = lns[cch]
        gl = gpool.tile([128, HC, 512], BF16, tag="g", name=f"gl{cch}",
                        bufs=2)
        for m in range(HC):
            o1 = ltpool.tile([128, 512], BF16, tag="lt", name="o1")
            nc.vector.tensor_mul(out=o1, in0=hs[cch][:, m, :], in1=rstd)
            o2 = ltpool.tile([128, 512], BF16, tag="lt", name="o2")
            nc.vector.tensor_add(out=o2, in0=o1, in1=bln)
            nc.scalar.activation(out=gl[:, m, :], in_=o2, func=AF.Gelu,
                                 bias=v["be_v"][:, m:m + 1],
                                 scale=v["ga_v"][:, m:m + 1])
        gls.append(gl)
    st["gls"] = gls


def stage_D1(nc, p, st, pst):
    """Windowed-attention context via banded matmuls; ctxT copies rotate
    over DVE/ACT/Pool to keep the 3-bank PSUM rotation drained."""
    ppool, ps_sm = p["persist"], p["ps_sm"]
    U = st["U"]
    ctxT = ppool.tile([128, HC, TOK], BF16, tag="ctxT", name="ctxT", bufs=1)
    st["ctxT"] = ctxT
    k = 0
    for half in range(2):
        for hc in range(HC):
            pc = ps_sm.tile([128, 512], F32, tag="sm", name="pc")
            banded(nc, pc, half,
                   lambda j: st["x_sb"][:, j, hc * 128:(hc + 1) * 128],
                   lambda j: U[j])
            dst = ctxT[:, hc, 512 * half:512 * half + 512]
            if k % 2:
                nc.scalar.copy(out=dst, in_=pc)
            else:
                nc.vector.tensor_copy(out=dst, in_=pc)
            k += 1


def stage_D2(nc, p, st, pst):
    """W1 for both 512-chunks with shared stationaries; h copies + hsq."""
    v = pviews(pst)
    hpool, sqpool, ps_mm = p["one"], p["one"], p["ps_mm"]
    hs, qs = [], []
    for cch in range(2):
        hs.append(hpool.tile([128, HC, 512], BF16, tag="h", name=f"h{cch}",
                             bufs=2))
        qs.append(sqpool.tile([128, HC, 512], BF16, tag="hsq",
                              name=f"q{cch}", bufs=2))
    for m in range(HC):
        ph0 = ps_mm.tile([128, 512], F32, tag="mm", name="ph0")
        ph1 = ps_mm.tile([128, 512], F32, tag="mm", name="ph1")
        for k in range(KC):
            for cch, ph in ((0, ph0), (1, ph1)):
                c0 = 512 * cch
                rhs = (st["xT"][:, k, M + c0:M + c0 + 512] if k < HC
                       else st["ctxT"][:, k - HC, c0:c0 + 512])
                nc.tensor.matmul(ph, v["w1_v"][:, k, m * 128:(m + 1) * 128],
                                 rhs, start=(k == 0), stop=(k == KC - 1))
        for cch, ph in ((0, ph0), (1, ph1)):
            if cch:
                nc.scalar.activation(out=hs[cch][:, m, :], in_=ph,
                                     func=AF.Identity,
                                     bias=v["b1_v"][:, m:m + 1])
            else:
                nc.vector.tensor_scalar_add(out=hs[cch][:, m, :], in0=ph,
                                            scalar1=v["b1_v"][:, m:m + 1])
            nc.vector.tensor_mul(out=qs[cch][:, m, :], in0=hs[cch][:, m, :],
                                 in1=hs[cch][:, m, :])
    st["hs"], st["qs"] = hs, qs


def stage_E(nc, p, st, pst):
    """W2 projection, 0.5*base + 0.5*ctx combine, transpose, store."""
    v = pviews(pst)
    ppool, ltpool = p["persist"], p["lt"]
    ps_mm, ps_st = p["ps_mm"], p["ps_st"]
    logitsT = ppool.tile([L, TOK], F32, tag="logitsT", name="logitsT")
    for cch in range(2):
        c0 = 512 * cch
        pl = ps_mm.tile([128, 512], F32, tag="mm", name="pl")
        for k in range(HC):
            nc.tensor.matmul(pl[:L, :], v["wst_v"][:, k, :],
                             st["gls"][cch][:, k, :],
                             start=(k == 0), stop=(k == HC - 1))
        blh = ltpool.tile([128, 512], F32, tag="blh", name="blh", bufs=2)
        nc.scalar.activation(out=blh[:L, :],
                             in_=st["sb10"][0:L, M + c0:M + c0 + 512],
                             func=AF.Identity, bias=v["bias9"], scale=0.5)
        nc.vector.scalar_tensor_tensor(out=logitsT[:, c0:c0 + 512],
                                       in0=pl[:L, :], scalar=0.5,
                                       in1=blh[:L, :],
                                       op0=ALU.mult, op1=ALU.add)
        po = ps_st.tile([128, 512], F32, tag="st", name="po")
        out_nat = ppool.tile([128, 4, L], F32, tag=f"onat{cch}",
                             name=f"onat{cch}")
        for j in range(4):
            jj = 4 * cch + j
            nc.tensor.transpose(po[:, j * L:(j + 1) * L],
                                logitsT[:, 128 * jj:128 * (jj + 1)],
                                v["id9"])
        if cch:
            nc.scalar.copy(out=out_nat,
                           in_=po[:, :4 * L].rearrange("p (j l) -> p j l",
                                                       l=L))
        else:
            nc.vector.tensor_copy(out=out_nat,
                                  in_=po[:, :4 * L].rearrange(
                                      "p (j l) -> p j l", l=L))
        nc.sync.dma_start(out=st["out_view"][:, 4 * cch:4 * cch + 4, :],
                          in_=out_nat)


def emit_steps(nc, p, io, n):
    """Emit n pipeline steps + drain; self-contained (fill from scratch)."""
    sts = [None] * n
    for i in range(n):
        sts[i] = stage_Aload(nc, p, io)
        cur = sts[i - 1] if i >= 1 else None   # freshest fully-loaded params
        if i >= 1:
            stage_B(nc, p, sts[i - 1], sts[i - 1])
        if i >= 2:
            stage_C1(nc, p, sts[i - 2], cur)
            stage_C2a(nc, p, sts[i - 2], cur)
        if i >= 1:
            stage_D1(nc, p, sts[i - 1], sts[i - 1])
        stage_Acomp(nc, p, sts[i])
        if i >= 2:
            stage_C2(nc, p, sts[i - 2], cur)
        if i >= 1:
            stage_D2(nc, p, sts[i - 1], sts[i - 1])
        if i >= 2:
            stage_E(nc, p, sts[i - 2], cur)
    # drain
    last = sts[n - 1]
    stage_B(nc, p, last, last)
    if n >= 2:
        stage_C1(nc, p, sts[n - 2], last)
        stage_C2a(nc, p, sts[n - 2], last)
    stage_D1(nc, p, last, last)
    if n >= 2:
        stage_C2(nc, p, sts[n - 2], last)
    stage_D2(nc, p, last, last)
    if n >= 2:
        stage_E(nc, p, sts[n - 2], last)
    stage_C1(nc, p, last, last)
    stage_C2a(nc, p, last, last)
    stage_C2(nc, p, last, last)
    stage_E(nc, p, last, last)


def build(rep=1, unroll=None):
    nc = bacc.Bacc("TRN2", target_bir_lowering=False, debug=False,
                   num_devices=8)

    xt_d = nc.dram_tensor("xt_loc", [128, HC * NJ * 128], BF16,
                          kind="ExternalInput").ap()
    x_d = nc.dram_tensor("x_loc", [NJ * 128, H], BF16,
                         kind="ExternalInput").ap()
    pb_d = nc.dram_tensor("pblob", [128, PB2], BF16,
                          kind="ExternalInput").ap()
    pf_d = nc.dram_tensor("pfblob", [128, PF], F32,
                          kind="ExternalInput").ap()
    out_d = nc.dram_tensor("out_loc", [TOK, L], F32,
                           kind="ExternalOutput").ap()

    io = (xt_d, x_d, pb_d, pf_d, out_d)

    with tile.TileContext(nc) as tc, ExitStack() as ctx:
        p = make_pools(tc, ctx)
        if rep == 1:
            emit_steps(nc, p, io, 1)
        else:
            if unroll is None:
                unroll = next(u for u in (UNROLL, 16, 8, 4, 2, 1)
                              if rep % u == 0)
            with tc.For_i(0, rep // unroll):
                emit_steps(nc, p, io, unroll)
    nc.compile()
    return nc


def make_host_inputs(sequence_output, Wc, bc, wa, ba, W1, b1, gamma, beta,
                     W2, b2):
    x = np.asarray(sequence_output, np.float32)
    bf = ml_dtypes.bfloat16

    pb = np.zeros((128, PB2), dtype=bf)
    w1 = np.asarray(W1, np.float32)
    pb[:, W1C:W1C + KC * H] = (
        w1.reshape(KC, 128, H).transpose(1, 0, 2).reshape(128, KC * H))
    pb[:, WSTC:WSTC + HC * L] = (
        np.asarray(W2, np.float32).reshape(HC, 128, L)
        .transpose(1, 0, 2).reshape(128, HC * L))
    wac = np.zeros((128, HC, 33), np.float32)
    wac[:, :, :L] = (np.asarray(Wc, np.float32)
                     .reshape(HC, 128, L).transpose(1, 0, 2))
    wac[:, :, 32] = np.asarray(wa, np.float32).reshape(HC, 128).T
    pb[:, WAC:WAC + HC * 33] = wac.reshape(128, HC * 33)
    pb[:, ONEC:ONEC + 128] = 1.0
    r_idx = np.arange(128)[:, None]
    u_idx = np.arange(UW)[None, :]
    pb[:, MSKC:MSKC + UW] = ((u_idx - r_idx >= 0) &
                             (u_idx - r_idx <= 2 * M)).astype(np.float32)

    pf_shared = np.zeros((128, PF), np.float32)
    pf_shared[:, B1C:B1C + HC] = np.asarray(b1, np.float32).reshape(HC, 128).T
    pf_shared[:, GAC:GAC + HC] = np.asarray(gamma, np.float32).reshape(HC, 128).T
    pf_shared[:, BEC:BEC + HC] = np.asarray(beta, np.float32).reshape(HC, 128).T
    pf_shared[:L, B9C] = np.asarray(bc, np.float32) + np.asarray(b2, np.float32)
    pf_shared[0, E0C] = 1.0
    pf_shared[32, E0C] = 1.0
    pf_shared[:L, ID9C:ID9C + L] = np.eye(L, dtype=np.float32)
    pf_shared[:, EPSC] = EPS
    # ba: softmax is shift-invariant, and scores feed nothing else -> drop it.

    in_maps = []
    for c in range(8):
        b, s0 = c // 2, TOK * (c % 2)
        x_loc = np.zeros((NJ * 128, H), dtype=bf)
        lo, hi = max(0, s0 - M), min(S, s0 + TOK + M)
        dst = lo - (s0 - M)
        x_loc[dst:dst + hi - lo] = x[b, lo:hi].astype(bf)
        # xT[p, hc, flat] = x_loc[flat, hc*128+p], flattened to [128, HC*1152]
        xt_loc = np.ascontiguousarray(
            x_loc.reshape(NJ * 128, HC, 128).transpose(2, 1, 0)
            .reshape(128, HC * NJ * 128))
        f = np.arange(128)[:, None] + 128 * np.arange(NJ)[None, :]
        g = s0 + f - M
        emask_np = ((g >= 0) & (g < S) & (f < FLAT)).astype(np.float32)
        pf_c = pf_shared.copy()
        pf_c[:, EMC:EMC + NJ] = emask_np
        in_maps.append({"xt_loc": xt_loc, "x_loc": x_loc, "pblob": pb,
                        "pfblob": pf_c})
    return in_maps


_cache = {}


def kernel(**inputs):
    if "nc" not in _cache:
        _cache["nc"] = build(rep=1)
    nc = _cache["nc"]
    in_maps = make_host_inputs(**inputs)
    res = run_bass_kernel_spmd(nc, in_maps, core_ids=list(range(8)))
    out = np.zeros((B, S, L), np.float32)
    for c in range(8):
        b, s0 = c // 2, TOK * (c % 2)
        out[b, s0:s0 + TOK] = res.results[c]["out_loc"]
    return out


# revision 20
# speedup vs baseline: 1.0145x; 1.0145x over previous
"""ContextAwareSpanClassifier Trainium2 Bass kernel (bf16, software-pipelined).

Problem (hardcoded): B=4, S=2048, H=768, L=9, M=5 (window W=11).
  base_logits = x @ Wc + bc
  s = x . wa + ba ; windowed softmax over [t-5, t+5] (seq-edge masked)
  ctx[t] = sum_o attn[t,o] * x[t+o]
  h = gelu_erf(LN(cat(x,ctx) @ W1 + b1) * gamma + beta)
  out = 0.5*base_logits + 0.5*(h @ W2 + b2)

Sharding: data parallel over B*S = 8192 tokens -> 8 cores x 1024 tokens
(core c: batch c//2, seq half (c%2)*1024) with 5-token zero-padded halos.
Params replicated. ba shift cancels in softmax and is otherwise unused.

Three-deep software pipeline: each unrolled step emits stages for three
bodies so the PE queue never waits on a cross-engine chain:
  step(i): Aload_i   loads for body i (xT XBAR transpose, params, x_sb)
           B_{i-1}   softmax denominator banded (PE) -> recip -> A-muls
           C1_{i-2}  LN stats matmuls (PE; covers B's recip/A DVE latency)
           D1_{i-1}  ctx banded (PE) + ctxT copies (DVE/ACT/Pool rotate)
           Acomp_i   scores sweep (PE; xT DMA has had a step of lead),
                     pt_sc transposes, softmax exp -> e_col -> U tiles
           C2_{i-2}  LN scalars (ln+exp rstd: shares the ACT exp table
                     with softmax -> only 2 table loads/step), apply, gelu
           D2_{i-1}  W1 both 512-chunks interleaved (each stationary
                     loaded once), h copies + hsq
           E_{i-2}   W2 proj, base+ctx combine, out transposes, store
PE stream/step: denom, stats, ctx, scores, W1, W2 -- every cross-engine
wait is covered by PE work from a neighboring body. 2 ACT table loads
per step (exp-set: softmax exp + LN ln/exp-rstd; gelu-set: gelu, with
Identity/Copy ops valid in any set). B/C/E read params from the newest
loaded generation (values identical) so pb/pf need only 2 buffers.
"""

from contextlib import ExitStack

import numpy as np
import ml_dtypes

import concourse.bass as bass
import concourse.tile as tile
from concourse import bacc, mybir
from concourse.bass_utils import run_bass_kernel_spmd

F32 = mybir.dt.float32
BF16 = mybir.dt.bfloat16
AF = mybir.ActivationFunctionType
ALU = mybir.AluOpType

B, S, H = 4, 2048, 768
L, M = 9, 5
TOK = 1024             # tokens per core
NT = 8                 # 128-token output tiles per core
NJ = 9                 # x storage tiles (tile 8 has 10 valid rows)
FLAT = TOK + 2 * M     # 1034
FPAD = 1040
HC = H // 128          # 6
KC = 2 * H // 128      # 12
WB = 2 * M + 1         # 11
UW = 138               # skew-band width: 128 + 10
EPS = 1e-5
UNROLL = 32

# bf16 param blob column offsets (small constants first so the leading DMA
# slice unblocks transposes/scores while the big W1 slices stream in)
WAC = 0                       # [128, HC*33] per-k [Wc | pad | wa@32] stationaries
ONEC = WAC + HC * 33          # [128, 128] ones
MSKC = ONEC + 128             # [128, UW] skew-band mask
W1C = MSKC + UW               # [128, KC*H] w1[p, k*H + m] = W1[k*128+p, m]
WSTC = W1C + KC * H           # [128, HC*L] W2 k-slabs (gl part only)
PB2 = WSTC + HC * L
SMALL = W1C                   # leading small-constant slice width

# f32 param blob column offsets
EMC = 0                       # [128, NJ] edge mask (per-core)
B1C = EMC + NJ                # [128, HC]
GAC = B1C + HC                # [128, HC]
BEC = GAC + HC                # [128, HC]
B9C = BEC + HC                # [:9] bias9 = bc + b2
E0C = B9C + 1                 # [128,1] e0 basis column
ID9C = E0C + 1                # [:9, 9] eye(9)
EPSC = ID9C + L               # [128,1] eps
PF = EPSC + 1

INV_H = 1.0 / H


def make_pools(tc, ctx):
    p = {}
    p["const"] = ctx.enter_context(tc.tile_pool(name="const", bufs=2))
    p["persist"] = ctx.enter_context(tc.tile_pool(name="persist", bufs=2))
    p["one"] = ctx.enter_context(tc.tile_pool(name="one", bufs=1))
    p["small"] = ctx.enter_context(tc.tile_pool(name="small", bufs=2))
    p["ln"] = ctx.enter_context(tc.tile_pool(name="ln", bufs=4))
    p["lt"] = ctx.enter_context(tc.tile_pool(name="lt", bufs=3))
    p["ps_mm"] = ctx.enter_context(tc.tile_pool(name="ps_mm", bufs=3, space="PSUM"))
    p["ps_sm"] = ctx.enter_context(tc.tile_pool(name="ps_sm", bufs=3, space="PSUM"))
    p["ps_st"] = ctx.enter_context(tc.tile_pool(name="ps_st", bufs=2, space="PSUM"))
    return p


def pviews(st):
    pb, pf = st["pb"], st["pf"]
    return {
        "w1_v": pb[:, W1C:W1C + KC * H].rearrange("p (k m) -> p k m", k=KC),
        "wst_v": pb[:, WSTC:WSTC + HC * L].rearrange("p (k l) -> p k l", k=HC),
        "wa_v": pb[:, WAC:WAC + HC * 33].rearrange("p (k c) -> p k c", k=HC),
        "ones_v": pb[:, ONEC:ONEC + 128],
        "msk_v": pb[:, MSKC:MSKC + UW],
        "emask": pf[:, EMC:EMC + NJ],
        "b1_v": pf[:, B1C:B1C + HC],
        "ga_v": pf[:, GAC:GAC + HC],
        "be_v": pf[:, BEC:BEC + HC],
        "bias9": pf[:L, B9C:B9C + 1],
        "e0": pf[:, E0C:E0C + 1],
        "id9": pf[:L, ID9C:ID9C + L],
        "eps_v": pf[:, EPSC:EPSC + 1],
    }


def stage_Aload(nc, p, io):
    """Issue all DMA loads for a new body; returns its state dict.

    xT arrives pre-transposed from the host (plain DMA, no XBAR); big
    transfers are spread across SP/ACT/DVE HWDGE queues so multiple DMA
    rings run in parallel."""
    (xt_d, x_d, pb_d, pf_d, out_d) = io
    ppool, cpool = p["persist"], p["const"]
    st = {}
    st["xT"] = ppool.tile([128, HC, NJ * 128], BF16, tag="xT", name="xT")
    xt_view = st["xT"].rearrange("p h f -> p (h f)")
    half = HC * NJ * 128 // 2
    nc.sync.dma_start(out=xt_view[:, :half], in_=xt_d[:, :half])
    nc.sync.dma_start(out=xt_view[:, half:], in_=xt_d[:, half:])
    st["pb"] = cpool.tile([128, PB2], BF16, tag="pb", name="pb")
    nc.scalar.dma_start(out=st["pb"][:, :SMALL], in_=pb_d[:, :SMALL])
    st["pf"] = cpool.tile([128, PF], F32, tag="pf", name="pf")
    nc.scalar.dma_start(out=st["pf"], in_=pf_d)
    st["x_sb"] = ppool.tile([128, NJ, H], BF16, tag="x", name="x_sb")
    x_view = x_d.rearrange("(j p) h -> p j h", p=128)
    for a, b in ((0, 5), (5, 9)):
        nc.gpsimd.dma_start(out=st["x_sb"][:, a:b, :], in_=x_view[:, a:b, :])
    for (a, b), eng in (((W1C, W1C + 4 * H), nc.scalar),
                        ((W1C + 4 * H, W1C + 8 * H), nc.gpsimd),
                        ((W1C + 8 * H, PB2), nc.sync)):
        eng.dma_start(out=st["pb"][:, a:b], in_=pb_d[:, a:b])
    st["out_view"] = out_d.rearrange("(j p) l -> p j l", p=128)
    return st


def stage_Acomp(nc, p, st):
    """Scores/base sweep + pt_sc transposes + softmax exp/e_col/U tiles."""
    v = pviews(st)
    ppool, spool = p["persist"], p["small"]
    ps_mm, ps_sm = p["ps_mm"], p["ps_sm"]

    sb10 = ppool.tile([33, FPAD], F32, tag="sb10", name="sb10", bufs=3)
    st["sb10"] = sb10
    pt_sc = ps_sm.tile([128, 512], F32, tag="sm", name="pt_sc")
    nc.vector.memset(pt_sc[:, :16], 0.0)

    for ci in range(3):
        c0, n = ((0, 512), (512, 512), (1024, 10))[ci]
        ps = ps_mm.tile([128, 512], F32, tag="mm", name="sc_ps")
        for hc in range(HC):
            nc.tensor.matmul(ps[:33, :n], v["wa_v"][:, hc, :],
                             st["xT"][:, hc, c0:c0 + n],
                             start=(hc == 0), stop=(hc == HC - 1))
        if ci % 2:
            nc.vector.tensor_copy(out=sb10[:, c0:c0 + n], in_=ps[:33, :n])
        else:
            nc.scalar.copy(out=sb10[:, c0:c0 + n], in_=ps[:33, :n])
    for j in range(NJ):
        n = 128 if j < NJ - 1 else 10
        nc.tensor.transpose(pt_sc[:n, j:j + 1],
                            sb10[32:33, 128 * j:128 * j + n],
                            v["e0"][32:33, :])

    e_tmp = spool.tile([128, NJ], F32, tag="etmp", name="e_tmp")
    nc.scalar.activation(out=e_tmp, in_=pt_sc[:, :NJ], func=AF.Exp)
    e_col = spool.tile([128, NJ], F32, tag="ecol", name="e_col")
    nc.vector.tensor_mul(out=e_col, in0=e_tmp, in1=v["emask"])
    U = [None] * NJ
    for j in range(NJ):
        u = spool.tile([128, UW], BF16, tag=f"u{j}", name=f"u{j}", bufs=2)
        nc.gpsimd.tensor_scalar_mul(out=u, in0=v["msk_v"],
                                    scalar1=e_col[:, j:j + 1])
        U[j] = u
    st["U"] = U


def banded(nc, out_bank, half, lhs_of_j, rhs_of_j):
    # out_bank covers dst cols [512*half, 512*half+512); src tile j
    # contributes dst cols [128j-10, 128j+128), 10-col overlaps accumulate
    # in PSUM (start only on the bank's first writer)
    base = 512 * half
    first = True
    for j in range(4 * half, 4 * half + 5):
        lo = max(128 * j - 10, 0)
        hi = min(128 * j + 128, TOK)
        lo, hi = max(lo, base), min(hi, base + 512)
        if lo >= hi:
            continue
        ua = lo - (128 * j - 10)
        nc.tensor.matmul(out_bank[:, lo - base:hi - base],
                         lhs_of_j(j), rhs_of_j(j)[:, ua:ua + hi - lo],
                         start=first, stop=(j == 4 * half + 4))
        first = False


def stage_B(nc, p, st, pst):
    """Softmax denominator banded (PE) -> reciprocal -> A-muls (in U)."""
    v = pviews(pst)
    ppool, ps_sm = p["persist"], p["ps_sm"]
    U = st["U"]
    r_rep = ppool.tile([128, TOK], BF16, tag="rrep", name="r_rep")
    st["r_rep"] = r_rep
    pds = []
    for half in range(2):
        pd = ps_sm.tile([128, 512], F32, tag="sm", name="pd")
        banded(nc, pd, half, lambda j: v["ones_v"], lambda j: U[j])
        pds.append(pd)
    with nc.allow_low_precision(reason="bf16 softmax denom recip"):
        for half in range(2):
            nc.vector.reciprocal(out=r_rep[:, 512 * half:512 * half + 512],
                                 in_=pds[half])
    # A_j = U_j * r built in place (disjoint column pieces per half)
    for half in range(2):
        base = 512 * half
        for j in range(4 * half, 4 * half + 5):
            glo = max(128 * j - 10, base)
            ghi = min(128 * j + 128, base + 512)
            if glo >= ghi:
                continue
            ua = glo - (128 * j - 10)
            nc.vector.tensor_mul(out=U[j][:, ua:ua + ghi - glo],
                                 in0=U[j][:, ua:ua + ghi - glo],
                                 in1=r_rep[:, glo:ghi])


def stage_C1(nc, p, st, pst):
    """LN stats matmuls on PE (sum h, sum h^2 per 512-chunk)."""
    v = pviews(pst)
    ps_st = p["ps_st"]
    hs, qs = st["hs"], st["qs"]
    st["ps_stats"] = []
    for cch in range(2):
        ps_s = ps_st.tile([128, 512], F32, tag="st", name="ps_s")
        for m in range(HC):
            nc.tensor.matmul(ps_s, v["ones_v"], hs[cch][:, m, :],
                             start=(m == 0), stop=(m == HC - 1))
        ps_q = ps_st.tile([128, 512], F32, tag="st", name="ps_q")
        for m in range(HC):
            nc.tensor.matmul(ps_q, v["ones_v"], qs[cch][:, m, :],
                             start=(m == 0), stop=(m == HC - 1))
        st["ps_stats"].append((ps_s, ps_q))


def stage_C2a(nc, p, st, pst):
    """Drain the four stats PSUM banks immediately (mu via ACT, var via
    DVE) so the bank rotation never waits on later-queued engine work."""
    lnpool = p["ln"]
    st["lnmv"] = []
    for cch in range(2):
        ps_s, ps_q = st["ps_stats"][cch]
        mu = lnpool.tile([128, 512], F32, tag="lnmu", name="mu", bufs=2)
        nc.scalar.activation(out=mu, in_=ps_s, func=AF.Copy, scale=INV_H)
        musq = lnpool.tile([128, 512], F32, tag="ln", name="musq")
        nc.vector.tensor_mul(out=musq, in0=mu, in1=mu)
        var = lnpool.tile([128, 512], F32, tag="ln", name="var")
        nc.vector.scalar_tensor_tensor(out=var, in0=ps_q, scalar=INV_H,
                                       in1=musq, op0=ALU.mult,
                                       op1=ALU.subtract)
        st["lnmv"].append((mu, var))


def stage_C2(nc, p, st, pst):
    """LN scalars: sqrt+recip rstd, bln (cheap; PSUM-adjacent)."""
    v = pviews(pst)
    lnpool = p["ln"]
    lns = []
    for cch in range(2):
        mu, var = st["lnmv"][cch]
        sd = lnpool.tile([128, 512], F32, tag="ln", name="sd")
        nc.scalar.activation(out=sd, in_=var, func=AF.Sqrt, bias=v["eps_v"])
        rstd = lnpool.tile([128, 512], BF16, tag="lnb", name="rstd")
        with nc.allow_low_precision(reason="bf16 rstd is within LN tolerance"):
            nc.vector.reciprocal(out=rstd, in_=sd)
        bln = lnpool.tile([128, 512], BF16, tag="lnb", name="bln")
        nc.vector.scalar_tensor_tensor(out=bln, in0=mu, scalar=-1.0,
                                       in1=rstd, op0=ALU.mult, op1=ALU.mult)
        lns.append((rstd, bln))
    st["lns"] = lns


def stage_C3(nc, p, st, pst):
    """Apply LN affine + gelu. Emitted AFTER the next body's W1 so this
    ACT/DVE backlog never delays W1's PSUM drains."""
    v = pviews(pst)
    ltpool, gpool = p["lt"], p["one"]
    hs = st["hs"]
    gls = []
    for cch in range(2):
        rstd, bln = st["lns"][cch]
        gl = gpool.tile([128, HC, 512], BF16, tag="g", name=f"gl{cch}",
                        bufs=2)
        for m in range(HC):
            o1 = ltpool.tile([128, 512], BF16, tag="lt", name="o1")
            nc.vector.tensor_mul(out=o1, in0=hs[cch][:, m, :], in1=rstd)
            o2 = ltpool.tile([128, 512], BF16, tag="lt", name="o2")
            nc.vector.tensor_add(out=o2, in0=o1, in1=bln)
            nc.scalar.activation(out=gl[:, m, :], in_=o2, func=AF.Gelu,
                                 bias=v["be_v"][:, m:m + 1],
                                 scale=v["ga_v"][:, m:m + 1])
        gls.append(gl)
    st["gls"] = gls


def stage_D1(nc, p, st, pst):
    """Windowed-attention context via banded matmuls; ctxT copies rotate
    over DVE/ACT/Pool to keep the 3-bank PSUM rotation drained."""
    ppool, ps_sm = p["persist"], p["ps_sm"]
    U = st["U"]
    ctxT = ppool.tile([128, HC, TOK], BF16, tag="ctxT", name="ctxT", bufs=1)
    st["ctxT"] = ctxT
    k = 0
    for half in range(2):
        for hc in range(HC):
            pc = ps_sm.tile([128, 512], F32, tag="sm", name="pc")
            banded(nc, pc, half,
                   lambda j: st["x_sb"][:, j, hc * 128:(hc + 1) * 128],
                   lambda j: U[j])
            dst = ctxT[:, hc, 512 * half:512 * half + 512]
            if k % 2:
                nc.scalar.copy(out=dst, in_=pc)
            else:
                nc.vector.tensor_copy(out=dst, in_=pc)
            k += 1


def stage_D2(nc, p, st, pst):
    """W1 for both 512-chunks with shared stationaries; h copies + hsq."""
    v = pviews(pst)
    hpool, sqpool, ps_mm = p["one"], p["one"], p["ps_mm"]
    hs, qs = [], []
    for cch in range(2):
        hs.append(hpool.tile([128, HC, 512], BF16, tag="h", name=f"h{cch}",
                             bufs=4))
        qs.append(sqpool.tile([128, HC, 512], BF16, tag="hsq",
                              name=f"q{cch}", bufs=2))
    for m in range(HC):
        ph0 = ps_mm.tile([128, 512], F32, tag="mm", name="ph0")
        ph1 = ps_mm.tile([128, 512], F32, tag="mm", name="ph1")
        for k in range(KC):
            for cch, ph in ((0, ph0), (1, ph1)):
                c0 = 512 * cch
                rhs = (st["xT"][:, k, M + c0:M + c0 + 512] if k < HC
                       else st["ctxT"][:, k - HC, c0:c0 + 512])
                nc.tensor.matmul(ph, v["w1_v"][:, k, m * 128:(m + 1) * 128],
                                 rhs, start=(k == 0), stop=(k == KC - 1))
        for cch, ph in ((0, ph0), (1, ph1)):
            if cch:
                nc.scalar.activation(out=hs[cch][:, m, :], in_=ph,
                                     func=AF.Identity,
                                     bias=v["b1_v"][:, m:m + 1])
            else:
                nc.vector.tensor_scalar_add(out=hs[cch][:, m, :], in0=ph,
                                            scalar1=v["b1_v"][:, m:m + 1])
            nc.vector.tensor_mul(out=qs[cch][:, m, :], in0=hs[cch][:, m, :],
                                 in1=hs[cch][:, m, :])
    st["hs"], st["qs"] = hs, qs


def stage_E(nc, p, st, pst):
    """W2 projection, 0.5*base + 0.5*ctx combine, transpose, store."""
    v = pviews(pst)
    ppool, ltpool = p["persist"], p["lt"]
    ps_mm, ps_st = p["ps_mm"], p["ps_st"]
    logitsT = ppool.tile([L, TOK], F32, tag="logitsT", name="logitsT",
                         bufs=1)
    for cch in range(2):
        c0 = 512 * cch
        pl = ps_mm.tile([128, 512], F32, tag="mm", name="pl")
        for k in range(HC):
            nc.tensor.matmul(pl[:L, :], v["wst_v"][:, k, :],
                             st["gls"][cch][:, k, :],
                             start=(k == 0), stop=(k == HC - 1))
        blh = ltpool.tile([128, 512], F32, tag="blh", name="blh", bufs=2)
        nc.scalar.activation(out=blh[:L, :],
                             in_=st["sb10"][0:L, M + c0:M + c0 + 512],
                             func=AF.Identity, bias=v["bias9"], scale=0.5)
        nc.vector.scalar_tensor_tensor(out=logitsT[:, c0:c0 + 512],
                                       in0=pl[:L, :], scalar=0.5,
                                       in1=blh[:L, :],
                                       op0=ALU.mult, op1=ALU.add)
        po = ps_st.tile([128, 512], F32, tag="st", name="po")
        out_nat = ppool.tile([128, 4, L], F32, tag=f"onat{cch}",
                             name=f"onat{cch}")
        for j in range(4):
            jj = 4 * cch + j
            nc.tensor.transpose(po[:, j * L:(j + 1) * L],
                                logitsT[:, 128 * jj:128 * (jj + 1)],
                                v["id9"])
        if cch:
            nc.scalar.copy(out=out_nat,
                           in_=po[:, :4 * L].rearrange("p (j l) -> p j l",
                                                       l=L))
        else:
            nc.vector.tensor_copy(out=out_nat,
                                  in_=po[:, :4 * L].rearrange(
                                      "p (j l) -> p j l", l=L))
        nc.sync.dma_start(out=st["out_view"][:, 4 * cch:4 * cch + 4, :],
                          in_=out_nat)


def emit_steps(nc, p, io, n):
    """Emit n pipeline steps + drain; self-contained (fill from scratch)."""
    sts = [None] * n
    for i in range(n):
        sts[i] = stage_Aload(nc, p, io)
        cur = sts[i - 1] if i >= 1 else None   # freshest fully-loaded params
        if i >= 1:
            stage_B(nc, p, sts[i - 1], sts[i - 1])
        if i >= 2:
            stage_C1(nc, p, sts[i - 2], cur)
            stage_C2a(nc, p, sts[i - 2], cur)
        if i >= 1:
            stage_D1(nc, p, sts[i - 1], sts[i - 1])
        stage_Acomp(nc, p, sts[i])
        if i >= 2:
            stage_C2(nc, p, sts[i - 2], cur)
        if i >= 1:
            stage_D2(nc, p, sts[i - 1], sts[i - 1])
        if i >= 2:
            stage_C3(nc, p, sts[i - 2], cur)
            stage_E(nc, p, sts[i - 2], cur)
    # drain
    last = sts[n - 1]
    stage_B(nc, p, last, last)
    if n >= 2:
        stage_C1(nc, p, sts[n - 2], last)
        stage_C2a(nc, p, sts[n - 2], last)
    stage_D1(nc, p, last, last)
    if n >= 2:
        stage_C2(nc, p, sts[n - 2], last)
    stage_D2(nc, p, last, last)
    if n >= 2:
        stage_C3(nc, p, sts[n - 2], last)
        stage_E(nc, p, sts[n - 2], last)
    stage_C1(nc, p, last, last)
    stage_C2a(nc, p, last, last)
    stage_C2(nc, p, last, last)
    stage_C3(nc, p, last, last)
    stage_E(nc, p, last, last)


def build(rep=1, unroll=None):
    nc = bacc.Bacc("TRN2", target_bir_lowering=False, debug=False,
                   num_devices=8)

    xt_d = nc.dram_tensor("xt_loc", [128, HC * NJ * 128], BF16,
                          kind="ExternalInput").ap()
    x_d = nc.dram_tensor("x_loc", [NJ * 128, H], BF16,
                         kind="ExternalInput").ap()
    pb_d = nc.dram_tensor("pblob", [128, PB2], BF16,
                          kind="ExternalInput").ap()
    pf_d = nc.dram_tensor("pfblob", [128, PF], F32,
                          kind="ExternalInput").ap()
    out_d = nc.dram_tensor("out_loc", [TOK, L], F32,
                           kind="ExternalOutput").ap()

    io = (xt_d, x_d, pb_d, pf_d, out_d)

    with tile.TileContext(nc) as tc, ExitStack() as ctx:
        p = make_pools(tc, ctx)
        if rep == 1:
            emit_steps(nc, p, io, 1)
        else:
            if unroll is None:
                unroll = next(u for u in (UNROLL, 16, 8, 4, 2, 1)
                              if rep % u == 0)
            with tc.For_i(0, rep // unroll):
                emit_steps(nc, p, io, unroll)
    nc.compile()
    return nc


def make_host_inputs(sequence_output, Wc, bc, wa, ba, W1, b1, gamma, beta,
                     W2, b2):
    x = np.asarray(sequence_output, np.float32)
    bf = ml_dtypes.bfloat16

    pb = np.zeros((128, PB2), dtype=bf)
    w1 = np.asarray(W1, np.float32)
    pb[:, W1C:W1C + KC * H] = (
        w1.reshape(KC, 128, H).transpose(1, 0, 2).reshape(128, KC * H))
    pb[:, WSTC:WSTC + HC * L] = (
        np.asarray(W2, np.float32).reshape(HC, 128, L)
        .transpose(1, 0, 2).reshape(128, HC * L))
    wac = np.zeros((128, HC, 33), np.float32)
    wac[:, :, :L] = (np.asarray(Wc, np.float32)
                     .reshape(HC, 128, L).transpose(1, 0, 2))
    wac[:, :, 32] = np.asarray(wa, np.float32).reshape(HC, 128).T
    pb[:, WAC:WAC + HC * 33] = wac.reshape(128, HC * 33)
    pb[:, ONEC:ONEC + 128] = 1.0
    r_idx = np.arange(128)[:, None]
    u_idx = np.arange(UW)[None, :]
    pb[:, MSKC:MSKC + UW] = ((u_idx - r_idx >= 0) &
                             (u_idx - r_idx <= 2 * M)).astype(np.float32)

    pf_shared = np.zeros((128, PF), np.float32)
    pf_shared[:, B1C:B1C + HC] = np.asarray(b1, np.float32).reshape(HC, 128).T
    pf_shared[:, GAC:GAC + HC] = np.asarray(gamma, np.float32).reshape(HC, 128).T
    pf_shared[:, BEC:BEC + HC] = np.asarray(beta, np.float32).reshape(HC, 128).T
    pf_shared[:L, B9C] = np.asarray(bc, np.float32) + np.asarray(b2, np.float32)
    pf_shared[0, E0C] = 1.0
    pf_shared[32, E0C] = 1.0
    pf_shared[:L, ID9C:ID9C + L] = np.eye(L, dtype=np.float32)
    pf_shared[:, EPSC] = EPS
    # ba: softmax is shift-invariant, and scores feed nothing else -> drop it.

    in_maps = []
    for c in range(8):
        b, s0 = c // 2, TOK * (c % 2)
        x_loc = np.zeros((NJ * 128, H), dtype=bf)
        lo, hi = max(0, s0 - M), min(S, s0 + TOK + M)
        dst = lo - (s0 - M)
        x_loc[dst:dst + hi - lo] = x[b, lo:hi].astype(bf)
        # xT[p, hc, flat] = x_loc[flat, hc*128+p], flattened to [128, HC*1152]
        xt_loc = np.ascontiguousarray(
            x_loc.reshape(NJ * 128, HC, 128).transpose(2, 1, 0)
            .reshape(128, HC * NJ * 128))
        f = np.arange(128)[:, None] + 128 * np.arange(NJ)[None, :]
        g = s0 + f - M
        emask_np = ((g >= 0) & (g < S) & (f < FLAT)).astype(np.float32)
        pf_c = pf_shared.copy()
        pf_c[:, EMC:EMC + NJ] = emask_np
        in_maps.append({"xt_loc": xt_loc, "x_loc": x_loc, "pblob": pb,
                        "pfblob": pf_c})
    return in_maps


_cache = {}


def kernel(**inputs):
    if "nc" not in _cache:
        _cache["nc"] = build(rep=1)
    nc = _cache["nc"]
    in_maps = make_host_inputs(**inputs)
    res = run_bass_kernel_spmd(nc, in_maps, core_ids=list(range(8)))
    out = np.zeros((B, S, L), np.float32)
    for c in range(8):
        b, s0 = c // 2, TOK * (c % 2)
        out[b, s0:s0 + TOK] = res.results[c]["out_loc"]
    return out


# revision 22
# speedup vs baseline: 1.1601x; 1.1435x over previous
"""ContextAwareSpanClassifier Trainium2 Bass kernel (bf16, software-pipelined).

Problem (hardcoded): B=4, S=2048, H=768, L=9, M=5 (window W=11).
  base_logits = x @ Wc + bc
  s = x . wa + ba ; windowed softmax over [t-5, t+5] (seq-edge masked)
  ctx[t] = sum_o attn[t,o] * x[t+o]
  h = gelu_erf(LN(cat(x,ctx) @ W1 + b1) * gamma + beta)
  out = 0.5*base_logits + 0.5*(h @ W2 + b2)

Sharding: data parallel over B*S = 8192 tokens -> 8 cores x 1024 tokens
(core c: batch c//2, seq half (c%2)*1024) with 5-token zero-padded halos.
Params replicated. ba shift cancels in softmax and is otherwise unused.

Three-deep software pipeline: each unrolled step emits stages for three
bodies so the PE queue never waits on a cross-engine chain:
  step(i): Aload_i   loads for body i (xT XBAR transpose, params, x_sb)
           B_{i-1}   softmax denominator banded (PE) -> recip -> A-muls
           C1_{i-2}  LN stats matmuls (PE; covers B's recip/A DVE latency)
           D1_{i-1}  ctx banded (PE) + ctxT copies (DVE/ACT/Pool rotate)
           Acomp_i   scores sweep (PE; xT DMA has had a step of lead),
                     pt_sc transposes, softmax exp -> e_col -> U tiles
           C2_{i-2}  LN scalars (ln+exp rstd: shares the ACT exp table
                     with softmax -> only 2 table loads/step), apply, gelu
           D2_{i-1}  W1 both 512-chunks interleaved (each stationary
                     loaded once), h copies + hsq
           E_{i-2}   W2 proj, base+ctx combine, out transposes, store
PE stream/step: denom, stats, ctx, scores, W1, W2 -- every cross-engine
wait is covered by PE work from a neighboring body. 2 ACT table loads
per step (exp-set: softmax exp + LN ln/exp-rstd; gelu-set: gelu, with
Identity/Copy ops valid in any set). B/C/E read params from the newest
loaded generation (values identical) so pb/pf need only 2 buffers.
"""

from contextlib import ExitStack

import numpy as np
import ml_dtypes

import concourse.bass as bass
import concourse.tile as tile
from concourse import bacc, mybir
from concourse.bass_utils import run_bass_kernel_spmd

F32 = mybir.dt.float32
BF16 = mybir.dt.bfloat16
AF = mybir.ActivationFunctionType
ALU = mybir.AluOpType

B, S, H = 4, 2048, 768
L, M = 9, 5
TOK = 1024             # tokens per core
NT = 8                 # 128-token output tiles per core
NJ = 9                 # x storage tiles (tile 8 has 10 valid rows)
FLAT = TOK + 2 * M     # 1034
FPAD = 1040
HC = H // 128          # 6
KC = 2 * H // 128      # 12
WB = 2 * M + 1         # 11
UW = 138               # skew-band width: 128 + 10
EPS = 1e-5
UNROLL = 64

# bf16 param blob column offsets (small constants first so the leading DMA
# slice unblocks transposes/scores while the big W1 slices stream in)
WAC = 0                       # [128, HC*33] per-k [Wc | pad | wa@32] stationaries
ONEC = WAC + HC * 33          # [128, 128] ones
MSKC = ONEC + 128             # [128, UW] skew-band mask
W1C = MSKC + UW               # [128, KC*H] w1[p, k*H + m] = W1[k*128+p, m]
WSTC = W1C + KC * H           # [128, HC*L] W2 k-slabs (gl part only)
PB2 = WSTC + HC * L
SMALL = W1C                   # leading small-constant slice width

# f32 param blob column offsets
EMC = 0                       # [128, NJ] edge mask (per-core)
B1C = EMC + NJ                # [128, HC]
GAC = B1C + HC                # [128, HC]
BEC = GAC + HC                # [128, HC]
B9C = BEC + HC                # [:9] bias9 = bc + b2
E0C = B9C + 1                 # [128,1] e0 basis column
ID9C = E0C + 1                # [:9, 9] eye(9)
EPSC = ID9C + L               # [128,1] eps
PF = EPSC + 1

INV_H = 1.0 / H


def make_pools(tc, ctx):
    p = {}
    p["const"] = ctx.enter_context(tc.tile_pool(name="const", bufs=2))
    p["persist"] = ctx.enter_context(tc.tile_pool(name="persist", bufs=2))
    p["one"] = ctx.enter_context(tc.tile_pool(name="one", bufs=1))
    p["small"] = ctx.enter_context(tc.tile_pool(name="small", bufs=2))
    p["ln"] = ctx.enter_context(tc.tile_pool(name="ln", bufs=4))
    p["lt"] = ctx.enter_context(tc.tile_pool(name="lt", bufs=3))
    p["ps_mm"] = ctx.enter_context(tc.tile_pool(name="ps_mm", bufs=3, space="PSUM"))
    p["ps_sm"] = ctx.enter_context(tc.tile_pool(name="ps_sm", bufs=3, space="PSUM"))
    p["ps_st"] = ctx.enter_context(tc.tile_pool(name="ps_st", bufs=2, space="PSUM"))
    return p


def pviews(st):
    pb, pf = st["pb"], st["pf"]
    return {
        "w1_v": pb[:, W1C:W1C + KC * H].rearrange("p (k m) -> p k m", k=KC),
        "wst_v": pb[:, WSTC:WSTC + HC * L].rearrange("p (k l) -> p k l", k=HC),
        "wa_v": pb[:, WAC:WAC + HC * 33].rearrange("p (k c) -> p k c", k=HC),
        "ones_v": pb[:, ONEC:ONEC + 128],
        "msk_v": pb[:, MSKC:MSKC + UW],
        "emask": pf[:, EMC:EMC + NJ],
        "b1_v": pf[:, B1C:B1C + HC],
        "ga_v": pf[:, GAC:GAC + HC],
        "be_v": pf[:, BEC:BEC + HC],
        "bias9": pf[:L, B9C:B9C + 1],
        "e0": pf[:, E0C:E0C + 1],
        "id9": pf[:L, ID9C:ID9C + L],
        "eps_v": pf[:, EPSC:EPSC + 1],
    }


def stage_Aload(nc, p, io):
    """Issue all DMA loads for a new body; returns its state dict.

    xT arrives pre-transposed from the host (plain DMA, no XBAR); big
    transfers are spread across SP/ACT/DVE HWDGE queues so multiple DMA
    rings run in parallel."""
    (xt_d, x_d, pb_d, pf_d, out_d) = io
    ppool, cpool = p["persist"], p["const"]
    st = {}
    st["xT"] = ppool.tile([128, HC, NJ * 128], BF16, tag="xT", name="xT")
    xt_view = st["xT"].rearrange("p h f -> p (h f)")
    half = HC * NJ * 128 // 2
    nc.sync.dma_start(out=xt_view[:, :half], in_=xt_d[:, :half])
    nc.sync.dma_start(out=xt_view[:, half:], in_=xt_d[:, half:])
    st["pb"] = cpool.tile([128, PB2], BF16, tag="pb", name="pb")
    nc.scalar.dma_start(out=st["pb"][:, :SMALL], in_=pb_d[:, :SMALL])
    st["pf"] = cpool.tile([128, PF], F32, tag="pf", name="pf")
    nc.scalar.dma_start(out=st["pf"], in_=pf_d)
    st["x_sb"] = ppool.tile([128, NJ, H], BF16, tag="x", name="x_sb")
    x_view = x_d.rearrange("(j p) h -> p j h", p=128)
    for a, b in ((0, 5), (5, 9)):
        nc.gpsimd.dma_start(out=st["x_sb"][:, a:b, :], in_=x_view[:, a:b, :])
    for (a, b), eng in (((W1C, W1C + 4 * H), nc.scalar),
                        ((W1C + 4 * H, W1C + 8 * H), nc.gpsimd),
                        ((W1C + 8 * H, PB2), nc.sync)):
        eng.dma_start(out=st["pb"][:, a:b], in_=pb_d[:, a:b])
    st["out_view"] = out_d.rearrange("(j p) l -> p j l", p=128)
    return st


def stage_Acomp(nc, p, st):
    """Scores/base sweep + pt_sc transposes + softmax exp/e_col/U tiles."""
    v = pviews(st)
    ppool, spool = p["persist"], p["small"]
    ps_mm, ps_sm = p["ps_mm"], p["ps_sm"]

    sb10 = ppool.tile([33, FPAD], F32, tag="sb10", name="sb10", bufs=3)
    st["sb10"] = sb10
    pt_sc = ps_sm.tile([128, 512], F32, tag="sm", name="pt_sc")
    nc.vector.memset(pt_sc[:, :16], 0.0)

    for ci in range(3):
        c0, n = ((0, 512), (512, 512), (1024, 10))[ci]
        ps = ps_mm.tile([128, 512], F32, tag="mm", name="sc_ps")
        for hc in range(HC):
            nc.tensor.matmul(ps[:33, :n], v["wa_v"][:, hc, :],
                             st["xT"][:, hc, c0:c0 + n],
                             start=(hc == 0), stop=(hc == HC - 1))
        if ci % 2:
            nc.vector.tensor_copy(out=sb10[:, c0:c0 + n], in_=ps[:33, :n])
        else:
            nc.scalar.copy(out=sb10[:, c0:c0 + n], in_=ps[:33, :n])
    for j in range(NJ):
        n = 128 if j < NJ - 1 else 10
        nc.tensor.transpose(pt_sc[:n, j:j + 1],
                            sb10[32:33, 128 * j:128 * j + n],
                            v["e0"][32:33, :])

    e_tmp = spool.tile([128, NJ], F32, tag="etmp", name="e_tmp")
    nc.scalar.activation(out=e_tmp, in_=pt_sc[:, :NJ], func=AF.Exp)
    e_col = spool.tile([128, NJ], F32, tag="ecol", name="e_col")
    nc.vector.tensor_mul(out=e_col, in0=e_tmp, in1=v["emask"])
    U = [None] * NJ
    for j in range(NJ):
        u = spool.tile([128, UW], BF16, tag=f"u{j}", name=f"u{j}", bufs=2)
        nc.gpsimd.tensor_scalar_mul(out=u, in0=v["msk_v"],
                                    scalar1=e_col[:, j:j + 1])
        U[j] = u
    st["U"] = U


def banded(nc, out_bank, half, lhs_of_j, rhs_of_j):
    # out_bank covers dst cols [512*half, 512*half+512); src tile j
    # contributes dst cols [128j-10, 128j+128), 10-col overlaps accumulate
    # in PSUM (start only on the bank's first writer)
    base = 512 * half
    first = True
    for j in range(4 * half, 4 * half + 5):
        lo = max(128 * j - 10, 0)
        hi = min(128 * j + 128, TOK)
        lo, hi = max(lo, base), min(hi, base + 512)
        if lo >= hi:
            continue
        ua = lo - (128 * j - 10)
        nc.tensor.matmul(out_bank[:, lo - base:hi - base],
                         lhs_of_j(j), rhs_of_j(j)[:, ua:ua + hi - lo],
                         start=first, stop=(j == 4 * half + 4))
        first = False


def stage_B(nc, p, st, pst):
    """Softmax denominator banded (PE) -> reciprocal -> A-muls (in U)."""
    v = pviews(pst)
    ppool, ps_sm = p["persist"], p["ps_sm"]
    U = st["U"]
    r_rep = ppool.tile([128, TOK], BF16, tag="rrep", name="r_rep")
    st["r_rep"] = r_rep
    pds = []
    for half in range(2):
        pd = ps_sm.tile([128, 512], F32, tag="sm", name="pd")
        banded(nc, pd, half, lambda j: v["ones_v"], lambda j: U[j])
        pds.append(pd)
    with nc.allow_low_precision(reason="bf16 softmax denom recip"):
        for half in range(2):
            nc.vector.reciprocal(out=r_rep[:, 512 * half:512 * half + 512],
                                 in_=pds[half])
    # A_j = U_j * r built in place (disjoint column pieces per half)
    for half in range(2):
        base = 512 * half
        for j in range(4 * half, 4 * half + 5):
            glo = max(128 * j - 10, base)
            ghi = min(128 * j + 128, base + 512)
            if glo >= ghi:
                continue
            ua = glo - (128 * j - 10)
            nc.vector.tensor_mul(out=U[j][:, ua:ua + ghi - glo],
                                 in0=U[j][:, ua:ua + ghi - glo],
                                 in1=r_rep[:, glo:ghi])


def stage_C1(nc, p, st, pst):
    """LN stats matmuls on PE (sum h, sum h^2 per 512-chunk)."""
    v = pviews(pst)
    ps_st = p["ps_st"]
    hs, qs = st["hs"], st["qs"]
    st["ps_stats"] = []
    for cch in range(2):
        ps_s = ps_st.tile([128, 512], F32, tag="st", name="ps_s")
        for m in range(HC):
            nc.tensor.matmul(ps_s, v["ones_v"], hs[cch][:, m, :],
                             start=(m == 0), stop=(m == HC - 1))
        ps_q = ps_st.tile([128, 512], F32, tag="st", name="ps_q")
        for m in range(HC):
            nc.tensor.matmul(ps_q, v["ones_v"], qs[cch][:, m, :],
                             start=(m == 0), stop=(m == HC - 1))
        st["ps_stats"].append((ps_s, ps_q))


def stage_C2a(nc, p, st, pst):
    """Drain the four stats PSUM banks immediately (mu via ACT, var via
    DVE) so the bank rotation never waits on later-queued engine work."""
    lnpool = p["ln"]
    st["lnmv"] = []
    for cch in range(2):
        ps_s, ps_q = st["ps_stats"][cch]
        mu = lnpool.tile([128, 512], F32, tag="lnmu", name="mu", bufs=2)
        nc.scalar.activation(out=mu, in_=ps_s, func=AF.Copy, scale=INV_H)
        musq = lnpool.tile([128, 512], F32, tag="ln", name="musq")
        nc.vector.tensor_mul(out=musq, in0=mu, in1=mu)
        var = lnpool.tile([128, 512], F32, tag="ln", name="var")
        nc.vector.scalar_tensor_tensor(out=var, in0=ps_q, scalar=INV_H,
                                       in1=musq, op0=ALU.mult,
                                       op1=ALU.subtract)
        st["lnmv"].append((mu, var))


def stage_C2(nc, p, st, pst):
    """LN scalars: sqrt+recip rstd, bln (cheap; PSUM-adjacent)."""
    v = pviews(pst)
    lnpool = p["ln"]
    lns = []
    for cch in range(2):
        mu, var = st["lnmv"][cch]
        sd = lnpool.tile([128, 512], F32, tag="ln", name="sd")
        nc.scalar.activation(out=sd, in_=var, func=AF.Sqrt, bias=v["eps_v"])
        rstd = lnpool.tile([128, 512], BF16, tag="lnb", name="rstd")
        with nc.allow_low_precision(reason="bf16 rstd is within LN tolerance"):
            nc.vector.reciprocal(out=rstd, in_=sd)
        bln = lnpool.tile([128, 512], BF16, tag="lnb", name="bln")
        nc.vector.scalar_tensor_tensor(out=bln, in0=mu, scalar=-1.0,
                                       in1=rstd, op0=ALU.mult, op1=ALU.mult)
        lns.append((rstd, bln))
    st["lns"] = lns


def stage_C3(nc, p, st, pst):
    """Apply LN affine + gelu. Emitted AFTER the next body's W1 so this
    ACT/DVE backlog never delays W1's PSUM drains."""
    v = pviews(pst)
    ltpool, gpool = p["lt"], p["one"]
    hs = st["hs"]
    gls = []
    for cch in range(2):
        rstd, bln = st["lns"][cch]
        gl = gpool.tile([128, HC, 512], BF16, tag="g", name=f"gl{cch}",
                        bufs=2)
        for m in range(HC):
            o1 = ltpool.tile([128, 512], BF16, tag="lt", name="o1")
            nc.vector.tensor_mul(out=o1, in0=hs[cch][:, m, :], in1=rstd)
            o2 = ltpool.tile([128, 512], BF16, tag="lt", name="o2")
            nc.vector.tensor_add(out=o2, in0=o1, in1=bln)
            nc.scalar.activation(out=gl[:, m, :], in_=o2, func=AF.Gelu,
                                 bias=v["be_v"][:, m:m + 1],
                                 scale=v["ga_v"][:, m:m + 1])
        gls.append(gl)
    st["gls"] = gls


def stage_D1(nc, p, st, pst):
    """Windowed-attention context via banded matmuls; ctxT copies rotate
    over DVE/ACT/Pool to keep the 3-bank PSUM rotation drained."""
    ppool, ps_sm = p["persist"], p["ps_sm"]
    U = st["U"]
    ctxT = ppool.tile([128, HC, TOK], BF16, tag="ctxT", name="ctxT", bufs=1)
    st["ctxT"] = ctxT
    k = 0
    for half in range(2):
        for hc in range(HC):
            pc = ps_sm.tile([128, 512], F32, tag="sm", name="pc")
            banded(nc, pc, half,
                   lambda j: st["x_sb"][:, j, hc * 128:(hc + 1) * 128],
                   lambda j: U[j])
            dst = ctxT[:, hc, 512 * half:512 * half + 512]
            if k % 2:
                nc.scalar.copy(out=dst, in_=pc)
            else:
                nc.vector.tensor_copy(out=dst, in_=pc)
            k += 1


def stage_D2(nc, p, st, pst):
    """W1 for both 512-chunks with shared stationaries; h copies + hsq."""
    v = pviews(pst)
    hpool, sqpool, ps_mm = p["one"], p["one"], p["ps_mm"]
    hs, qs = [], []
    for cch in range(2):
        hs.append(hpool.tile([128, HC, 512], BF16, tag="h", name=f"h{cch}",
                             bufs=4))
        qs.append(sqpool.tile([128, HC, 512], BF16, tag="hsq",
                              name=f"q{cch}", bufs=2))
    for m in range(HC):
        ph0 = ps_mm.tile([128, 512], F32, tag="mm", name="ph0")
        ph1 = ps_mm.tile([128, 512], F32, tag="mm", name="ph1")
        for k in range(KC):
            for cch, ph in ((0, ph0), (1, ph1)):
                c0 = 512 * cch
                rhs = (st["xT"][:, k, M + c0:M + c0 + 512] if k < HC
                       else st["ctxT"][:, k - HC, c0:c0 + 512])
                nc.tensor.matmul(ph, v["w1_v"][:, k, m * 128:(m + 1) * 128],
                                 rhs, start=(k == 0), stop=(k == KC - 1))
        for cch, ph in ((0, ph0), (1, ph1)):
            if cch:
                nc.scalar.activation(out=hs[cch][:, m, :], in_=ph,
                                     func=AF.Identity,
                                     bias=v["b1_v"][:, m:m + 1])
            else:
                nc.vector.tensor_scalar_add(out=hs[cch][:, m, :], in0=ph,
                                            scalar1=v["b1_v"][:, m:m + 1])
            nc.vector.tensor_mul(out=qs[cch][:, m, :], in0=hs[cch][:, m, :],
                                 in1=hs[cch][:, m, :])
    st["hs"], st["qs"] = hs, qs


def stage_E(nc, p, st, pst):
    """W2 projection, 0.5*base + 0.5*ctx combine, transpose, store."""
    v = pviews(pst)
    ppool, ltpool = p["persist"], p["lt"]
    ps_mm, ps_st = p["ps_mm"], p["ps_st"]
    logitsT = ppool.tile([L, TOK], F32, tag="logitsT", name="logitsT",
                         bufs=1)
    for cch in range(2):
        c0 = 512 * cch
        pl = ps_mm.tile([128, 512], F32, tag="mm", name="pl")
        for k in range(HC):
            nc.tensor.matmul(pl[:L, :], v["wst_v"][:, k, :],
                             st["gls"][cch][:, k, :],
                             start=(k == 0), stop=(k == HC - 1))
        blh = ltpool.tile([128, 512], F32, tag="blh", name="blh", bufs=2)
        nc.scalar.activation(out=blh[:L, :],
                             in_=st["sb10"][0:L, M + c0:M + c0 + 512],
                             func=AF.Identity, bias=v["bias9"], scale=0.5)
        nc.vector.scalar_tensor_tensor(out=logitsT[:, c0:c0 + 512],
                                       in0=pl[:L, :], scalar=0.5,
                                       in1=blh[:L, :],
                                       op0=ALU.mult, op1=ALU.add)
        po = ps_st.tile([128, 512], F32, tag="st", name="po")
        out_nat = ppool.tile([128, 4, L], F32, tag=f"onat{cch}",
                             name=f"onat{cch}")
        for j in range(4):
            jj = 4 * cch + j
            nc.tensor.transpose(po[:, j * L:(j + 1) * L],
                                logitsT[:, 128 * jj:128 * (jj + 1)],
                                v["id9"])
        if cch:
            nc.scalar.copy(out=out_nat,
                           in_=po[:, :4 * L].rearrange("p (j l) -> p j l",
                                                       l=L))
        else:
            nc.vector.tensor_copy(out=out_nat,
                                  in_=po[:, :4 * L].rearrange(
                                      "p (j l) -> p j l", l=L))
        nc.sync.dma_start(out=st["out_view"][:, 4 * cch:4 * cch + 4, :],
                          in_=out_nat)


def emit_steps(nc, p, io, n):
    """Emit n pipeline steps + drain; self-contained (fill from scratch)."""
    sts = [None] * n
    for i in range(n):
        sts[i] = stage_Aload(nc, p, io)
        cur = sts[i - 1] if i >= 1 else None   # freshest fully-loaded params
        if i >= 1:
            stage_B(nc, p, sts[i - 1], sts[i - 1])
        if i >= 2:
            stage_C1(nc, p, sts[i - 2], cur)
            stage_C2a(nc, p, sts[i - 2], cur)
        if i >= 1:
            stage_D1(nc, p, sts[i - 1], sts[i - 1])
        stage_Acomp(nc, p, sts[i])
        if i >= 2:
            stage_C2(nc, p, sts[i - 2], cur)
        if i >= 1:
            stage_D2(nc, p, sts[i - 1], sts[i - 1])
        if i >= 2:
            stage_C3(nc, p, sts[i - 2], cur)
            stage_E(nc, p, sts[i - 2], cur)
    # drain
    last = sts[n - 1]
    stage_B(nc, p, last, last)
    if n >= 2:
        stage_C1(nc, p, sts[n - 2], last)
        stage_C2a(nc, p, sts[n - 2], last)
    stage_D1(nc, p, last, last)
    if n >= 2:
        stage_C2(nc, p, sts[n - 2], last)
    stage_D2(nc, p, last, last)
    if n >= 2:
        stage_C3(nc, p, sts[n - 2], last)
        stage_E(nc, p, sts[n - 2], last)
    stage_C1(nc, p, last, last)
    stage_C2a(nc, p, last, last)
    stage_C2(nc, p, last, last)
    stage_C3(nc, p, last, last)
    stage_E(nc, p, last, last)


def build(rep=1, unroll=None):
    nc = bacc.Bacc("TRN2", target_bir_lowering=False, debug=False,
                   num_devices=8)

    xt_d = nc.dram_tensor("xt_loc", [128, HC * NJ * 128], BF16,
                          kind="ExternalInput").ap()
    x_d = nc.dram_tensor("x_loc", [NJ * 128, H], BF16,
                         kind="ExternalInput").ap()
    pb_d = nc.dram_tensor("pblob", [128, PB2], BF16,
                          kind="ExternalInput").ap()
    pf_d = nc.dram_tensor("pfblob", [128, PF], F32,
                          kind="ExternalInput").ap()
    out_d = nc.dram_tensor("out_loc", [TOK, L], F32,
                           kind="ExternalOutput").ap()

    io = (xt_d, x_d, pb_d, pf_d, out_d)

    with tile.TileContext(nc) as tc, ExitStack() as ctx:
        p = make_pools(tc, ctx)
        if rep == 1:
            emit_steps(nc, p, io, 1)
        else:
            if unroll is None:
                unroll = next(u for u in (UNROLL, 32, 16, 8, 4, 2, 1)
                              if rep % u == 0)
            with tc.For_i(0, rep // unroll):
                emit_steps(nc, p, io, unroll)
    nc.compile()
    return nc


def make_host_inputs(sequence_output, Wc, bc, wa, ba, W1, b1, gamma, beta,
                     W2, b2):
    x = np.asarray(sequence_output, np.float32)
    bf = ml_dtypes.bfloat16

    pb = np.zeros((128, PB2), dtype=bf)
    w1 = np.asarray(W1, np.float32)
    pb[:, W1C:W1C + KC * H] = (
        w1.reshape(KC, 128, H).transpose(1, 0, 2).reshape(128, KC * H))
    pb[:, WSTC:WSTC + HC * L] = (
        np.asarray(W2, np.float32).reshape(HC, 128, L)
        .transpose(1, 0, 2).reshape(128, HC * L))
    wac = np.zeros((128, HC, 33), np.float32)
    wac[:, :, :L] = (np.asarray(Wc, np.float32)
                     .reshape(HC, 128, L).transpose(1, 0, 2))
    wac[:, :, 32] = np.asarray(wa, np.float32).reshape(HC, 128).T
    pb[:, WAC:WAC + HC * 33] = wac.reshape(128, HC * 33)
    pb[:, ONEC:ONEC + 128] = 1.0
    r_idx = np.arange(128)[:, None]
    u_idx = np.arange(UW)[None, :]
    pb[:, MSKC:MSKC + UW] = ((u_idx - r_idx >= 0) &
                             (u_idx - r_idx <= 2 * M)).astype(np.float32)

    pf_shared = np.zeros((128, PF), np.float32)
    pf_shared[:, B1C:B1C + HC] = np.asarray(b1, np.float32).reshape(HC, 128).T
    pf_shared[:, GAC:GAC + HC] = np.asarray(gamma, np.float32).reshape(HC, 128).T
    pf_shared[:, BEC:BEC + HC] = np.asarray(beta, np.float32).reshape(HC, 128).T
    pf_shared[:L, B9C] = np.asarray(bc, np.float32) + np.asarray(b2, np.float32)
    pf_shared[0, E0C] = 1.0
    pf_shared[32, E0C] = 1.0
    pf_shared[:L, ID9C:ID9C + L] = np.eye(L, dtype=np.float32)
    pf_shared[:, EPSC] = EPS
    # ba: softmax is shift-invariant, and scores feed nothing else -> drop it.

    in_maps = []
    for c in range(8):
        b, s0 = c // 2, TOK * (c % 2)
        x_loc = np.zeros((NJ * 128, H), dtype=bf)
        lo, hi = max(0, s0 - M), min(S, s0 + TOK + M)
        dst = lo - (s0 - M)
        x_loc[dst:dst + hi - lo] = x[b, lo:hi].astype(bf)
        # xT[p, hc, flat] = x_loc[flat, hc*128+p], flattened to [128, HC*1152]
        xt_loc = np.ascontiguousarray(
            x_loc.reshape(NJ * 128, HC, 128).transpose(2, 1, 0)
            .reshape(128, HC * NJ * 128))
        f = np.arange(128)[:, None] + 128 * np.arange(NJ)[None, :]
        g = s0 + f - M
        emask_np = ((g >= 0) & (g < S) & (f < FLAT)).astype(np.float32)
        pf_c = pf_shared.copy()
        pf_c[:, EMC:EMC + NJ] = emask_np
        in_maps.append({"xt_loc": xt_loc, "x_loc": x_loc, "pblob": pb,
                        "pfblob": pf_c})
    return in_maps


_cache = {}


def kernel(**inputs):
    if "nc" not in _cache:
        _cache["nc"] = build(rep=1)
    nc = _cache["nc"]
    in_maps = make_host_inputs(**inputs)
    res = run_bass_kernel_spmd(nc, in_maps, core_ids=list(range(8)))
    out = np.zeros((B, S, L), np.float32)
    for c in range(8):
        b, s0 = c // 2, TOK * (c % 2)
        out[b, s0:s0 + TOK] = res.results[c]["out_loc"]
    return out


# revision 24
# speedup vs baseline: 1.1667x; 1.0057x over previous
"""ContextAwareSpanClassifier Trainium2 Bass kernel (bf16, software-pipelined).

Problem (hardcoded): B=4, S=2048, H=768, L=9, M=5 (window W=11).
  base_logits = x @ Wc + bc
  s = x . wa + ba ; windowed softmax over [t-5, t+5] (seq-edge masked)
  ctx[t] = sum_o attn[t,o] * x[t+o]
  h = gelu_erf(LN(cat(x,ctx) @ W1 + b1) * gamma + beta)
  out = 0.5*base_logits + 0.5*(h @ W2 + b2)

Sharding: data parallel over B*S = 8192 tokens -> 8 cores x 1024 tokens
(core c: batch c//2, seq half (c%2)*1024) with 5-token zero-padded halos.
Params replicated. ba shift cancels in softmax and is otherwise unused.

Three-deep software pipeline: each unrolled step emits stages for three
bodies so the PE queue never waits on a cross-engine chain:
  step(i): Aload_i   loads for body i (xT XBAR transpose, params, x_sb)
           B_{i-1}   softmax denominator banded (PE) -> recip -> A-muls
           C1_{i-2}  LN stats matmuls (PE; covers B's recip/A DVE latency)
           D1_{i-1}  ctx banded (PE) + ctxT copies (DVE/ACT/Pool rotate)
           Acomp_i   scores sweep (PE; xT DMA has had a step of lead),
                     pt_sc transposes, softmax exp -> e_col -> U tiles
           C2_{i-2}  LN scalars (ln+exp rstd: shares the ACT exp table
                     with softmax -> only 2 table loads/step), apply, gelu
           D2_{i-1}  W1 both 512-chunks interleaved (each stationary
                     loaded once), h copies + hsq
           E_{i-2}   W2 proj, base+ctx combine, out transposes, store
PE stream/step: denom, stats, ctx, scores, W1, W2 -- every cross-engine
wait is covered by PE work from a neighboring body. 2 ACT table loads
per step (exp-set: softmax exp + LN ln/exp-rstd; gelu-set: gelu, with
Identity/Copy ops valid in any set). B/C/E read params from the newest
loaded generation (values identical) so pb/pf need only 2 buffers.
"""

from contextlib import ExitStack

import numpy as np
import ml_dtypes

import concourse.bass as bass
import concourse.tile as tile
from concourse import bacc, mybir
from concourse.bass_utils import run_bass_kernel_spmd

F32 = mybir.dt.float32
BF16 = mybir.dt.bfloat16
AF = mybir.ActivationFunctionType
ALU = mybir.AluOpType

B, S, H = 4, 2048, 768
L, M = 9, 5
TOK = 1024             # tokens per core
NT = 8                 # 128-token output tiles per core
NJ = 9                 # x storage tiles (tile 8 has 10 valid rows)
FLAT = TOK + 2 * M     # 1034
FPAD = 1040
HC = H // 128          # 6
KC = 2 * H // 128      # 12
WB = 2 * M + 1         # 11
UW = 138               # skew-band width: 128 + 10
EPS = 1e-5
UNROLL = 128

# bf16 param blob column offsets (small constants first so the leading DMA
# slice unblocks transposes/scores while the big W1 slices stream in)
WAC = 0                       # [128, HC*33] per-k [Wc | pad | wa@32] stationaries
ONEC = WAC + HC * 33          # [128, 128] ones
MSKC = ONEC + 128             # [128, UW] skew-band mask
W1C = MSKC + UW               # [128, KC*H] w1[p, k*H + m] = W1[k*128+p, m]
WSTC = W1C + KC * H           # [128, HC*L] W2 k-slabs (gl part only)
PB2 = WSTC + HC * L
SMALL = W1C                   # leading small-constant slice width

# f32 param blob column offsets
EMC = 0                       # [128, NJ] edge mask (per-core)
B1C = EMC + NJ                # [128, HC]
GAC = B1C + HC                # [128, HC]
BEC = GAC + HC                # [128, HC]
B9C = BEC + HC                # [:9] bias9 = bc + b2
E0C = B9C + 1                 # [128,1] e0 basis column
ID9C = E0C + 1                # [:9, 9] eye(9)
EPSC = ID9C + L               # [128,1] eps
PF = EPSC + 1

INV_H = 1.0 / H


def make_pools(tc, ctx):
    p = {}
    p["const"] = ctx.enter_context(tc.tile_pool(name="const", bufs=2))
    p["persist"] = ctx.enter_context(tc.tile_pool(name="persist", bufs=2))
    p["one"] = ctx.enter_context(tc.tile_pool(name="one", bufs=1))
    p["small"] = ctx.enter_context(tc.tile_pool(name="small", bufs=2))
    p["ln"] = ctx.enter_context(tc.tile_pool(name="ln", bufs=4))
    p["lt"] = ctx.enter_context(tc.tile_pool(name="lt", bufs=3))
    p["ps_mm"] = ctx.enter_context(tc.tile_pool(name="ps_mm", bufs=3, space="PSUM"))
    p["ps_sm"] = ctx.enter_context(tc.tile_pool(name="ps_sm", bufs=3, space="PSUM"))
    p["ps_st"] = ctx.enter_context(tc.tile_pool(name="ps_st", bufs=2, space="PSUM"))
    return p


def pviews(st):
    pb, pf = st["pb"], st["pf"]
    return {
        "w1_v": pb[:, W1C:W1C + KC * H].rearrange("p (k m) -> p k m", k=KC),
        "wst_v": pb[:, WSTC:WSTC + HC * L].rearrange("p (k l) -> p k l", k=HC),
        "wa_v": pb[:, WAC:WAC + HC * 33].rearrange("p (k c) -> p k c", k=HC),
        "ones_v": pb[:, ONEC:ONEC + 128],
        "msk_v": pb[:, MSKC:MSKC + UW],
        "emask": pf[:, EMC:EMC + NJ],
        "b1_v": pf[:, B1C:B1C + HC],
        "ga_v": pf[:, GAC:GAC + HC],
        "be_v": pf[:, BEC:BEC + HC],
        "bias9": pf[:L, B9C:B9C + 1],
        "e0": pf[:, E0C:E0C + 1],
        "id9": pf[:L, ID9C:ID9C + L],
        "eps_v": pf[:, EPSC:EPSC + 1],
    }


def stage_Aload(nc, p, io):
    """Issue all DMA loads for a new body; returns its state dict.

    xT arrives pre-transposed from the host (plain DMA, no XBAR); big
    transfers are spread across SP/ACT/DVE HWDGE queues so multiple DMA
    rings run in parallel."""
    (xt_d, x_d, pb_d, pf_d, out_d) = io
    ppool, cpool = p["persist"], p["const"]
    st = {}
    st["xT"] = ppool.tile([128, HC, NJ * 128], BF16, tag="xT", name="xT")
    xt_view = st["xT"].rearrange("p h f -> p (h f)")
    half = HC * NJ * 128 // 2
    nc.sync.dma_start(out=xt_view[:, :half], in_=xt_d[:, :half])
    nc.sync.dma_start(out=xt_view[:, half:], in_=xt_d[:, half:])
    st["pb"] = cpool.tile([128, PB2], BF16, tag="pb", name="pb")
    nc.scalar.dma_start(out=st["pb"][:, :SMALL], in_=pb_d[:, :SMALL])
    st["pf"] = cpool.tile([128, PF], F32, tag="pf", name="pf")
    nc.scalar.dma_start(out=st["pf"], in_=pf_d)
    st["x_sb"] = ppool.tile([128, NJ, H], BF16, tag="x", name="x_sb")
    x_view = x_d.rearrange("(j p) h -> p j h", p=128)
    for a, b in ((0, 5), (5, 9)):
        nc.gpsimd.dma_start(out=st["x_sb"][:, a:b, :], in_=x_view[:, a:b, :])
    for (a, b), eng in (((W1C, W1C + 4 * H), nc.scalar),
                        ((W1C + 4 * H, W1C + 8 * H), nc.gpsimd),
                        ((W1C + 8 * H, PB2), nc.sync)):
        eng.dma_start(out=st["pb"][:, a:b], in_=pb_d[:, a:b])
    st["out_view"] = out_d.rearrange("(j p) l -> p j l", p=128)
    return st


def stage_Acomp(nc, p, st):
    """Scores/base sweep + pt_sc transposes + softmax exp/e_col/U tiles."""
    v = pviews(st)
    ppool, spool = p["persist"], p["small"]
    ps_mm, ps_sm = p["ps_mm"], p["ps_sm"]

    sb10 = ppool.tile([33, FPAD], F32, tag="sb10", name="sb10", bufs=3)
    st["sb10"] = sb10
    pt_sc = ps_sm.tile([128, 512], F32, tag="sm", name="pt_sc")
    nc.vector.memset(pt_sc[:, :16], 0.0)

    for ci in range(3):
        c0, n = ((0, 512), (512, 512), (1024, 10))[ci]
        ps = ps_mm.tile([128, 512], F32, tag="mm", name="sc_ps")
        for hc in range(HC):
            nc.tensor.matmul(ps[:33, :n], v["wa_v"][:, hc, :],
                             st["xT"][:, hc, c0:c0 + n],
                             start=(hc == 0), stop=(hc == HC - 1))
        if ci % 2:
            nc.vector.tensor_copy(out=sb10[:, c0:c0 + n], in_=ps[:33, :n])
        else:
            nc.scalar.copy(out=sb10[:, c0:c0 + n], in_=ps[:33, :n])
    for j in range(NJ):
        n = 128 if j < NJ - 1 else 10
        nc.tensor.transpose(pt_sc[:n, j:j + 1],
                            sb10[32:33, 128 * j:128 * j + n],
                            v["e0"][32:33, :])

    e_tmp = spool.tile([128, NJ], F32, tag="etmp", name="e_tmp")
    nc.scalar.activation(out=e_tmp, in_=pt_sc[:, :NJ], func=AF.Exp)
    e_col = spool.tile([128, NJ], F32, tag="ecol", name="e_col")
    nc.vector.tensor_mul(out=e_col, in0=e_tmp, in1=v["emask"])
    U = [None] * NJ
    for j in range(NJ):
        u = spool.tile([128, UW], BF16, tag=f"u{j}", name=f"u{j}", bufs=2)
        nc.gpsimd.tensor_scalar_mul(out=u, in0=v["msk_v"],
                                    scalar1=e_col[:, j:j + 1])
        U[j] = u
    st["U"] = U


def banded(nc, out_bank, half, lhs_of_j, rhs_of_j):
    # out_bank covers dst cols [512*half, 512*half+512); src tile j
    # contributes dst cols [128j-10, 128j+128), 10-col overlaps accumulate
    # in PSUM (start only on the bank's first writer)
    base = 512 * half
    first = True
    for j in range(4 * half, 4 * half + 5):
        lo = max(128 * j - 10, 0)
        hi = min(128 * j + 128, TOK)
        lo, hi = max(lo, base), min(hi, base + 512)
        if lo >= hi:
            continue
        ua = lo - (128 * j - 10)
        nc.tensor.matmul(out_bank[:, lo - base:hi - base],
                         lhs_of_j(j), rhs_of_j(j)[:, ua:ua + hi - lo],
                         start=first, stop=(j == 4 * half + 4))
        first = False


def stage_B(nc, p, st, pst):
    """Softmax denominator banded (PE) -> reciprocal -> A-muls (in U)."""
    v = pviews(pst)
    ppool, ps_sm = p["persist"], p["ps_sm"]
    U = st["U"]
    r_rep = ppool.tile([128, TOK], BF16, tag="rrep", name="r_rep")
    st["r_rep"] = r_rep
    pds = []
    for half in range(2):
        pd = ps_sm.tile([128, 512], F32, tag="sm", name="pd")
        banded(nc, pd, half, lambda j: v["ones_v"], lambda j: U[j])
        pds.append(pd)
    with nc.allow_low_precision(reason="bf16 softmax denom recip"):
        for half in range(2):
            nc.vector.reciprocal(out=r_rep[:, 512 * half:512 * half + 512],
                                 in_=pds[half])
    # A_j = U_j * r built in place (disjoint column pieces per half)
    for half in range(2):
        base = 512 * half
        for j in range(4 * half, 4 * half + 5):
            glo = max(128 * j - 10, base)
            ghi = min(128 * j + 128, base + 512)
            if glo >= ghi:
                continue
            ua = glo - (128 * j - 10)
            nc.vector.tensor_mul(out=U[j][:, ua:ua + ghi - glo],
                                 in0=U[j][:, ua:ua + ghi - glo],
                                 in1=r_rep[:, glo:ghi])


def stage_C1(nc, p, st, pst):
    """LN stats matmuls on PE (sum h, sum h^2 per 512-chunk)."""
    v = pviews(pst)
    ps_st = p["ps_st"]
    hs, qs = st["hs"], st["qs"]
    st["ps_stats"] = []
    for cch in range(2):
        ps_s = ps_st.tile([128, 512], F32, tag="st", name="ps_s")
        for m in range(HC):
            nc.tensor.matmul(ps_s, v["ones_v"], hs[cch][:, m, :],
                             start=(m == 0), stop=(m == HC - 1))
        ps_q = ps_st.tile([128, 512], F32, tag="st", name="ps_q")
        for m in range(HC):
            nc.tensor.matmul(ps_q, v["ones_v"], qs[cch][:, m, :],
                             start=(m == 0), stop=(m == HC - 1))
        st["ps_stats"].append((ps_s, ps_q))


def stage_C2a(nc, p, st, pst):
    """Drain the four stats PSUM banks immediately (mu via ACT, var via
    DVE) so the bank rotation never waits on later-queued engine work."""
    lnpool = p["ln"]
    st["lnmv"] = []
    for cch in range(2):
        ps_s, ps_q = st["ps_stats"][cch]
        mu = lnpool.tile([128, 512], F32, tag="lnmu", name="mu", bufs=2)
        nc.scalar.activation(out=mu, in_=ps_s, func=AF.Copy, scale=INV_H)
        musq = lnpool.tile([128, 512], F32, tag="ln", name="musq")
        nc.vector.tensor_mul(out=musq, in0=mu, in1=mu)
        var = lnpool.tile([128, 512], F32, tag="ln", name="var")
        nc.vector.scalar_tensor_tensor(out=var, in0=ps_q, scalar=INV_H,
                                       in1=musq, op0=ALU.mult,
                                       op1=ALU.subtract)
        st["lnmv"].append((mu, var))


def stage_C2(nc, p, st, pst):
    """LN scalars: sqrt+recip rstd, bln (cheap; PSUM-adjacent)."""
    v = pviews(pst)
    lnpool = p["ln"]
    lns = []
    for cch in range(2):
        mu, var = st["lnmv"][cch]
        sd = lnpool.tile([128, 512], F32, tag="ln", name="sd")
        nc.scalar.activation(out=sd, in_=var, func=AF.Sqrt, bias=v["eps_v"])
        rstd = lnpool.tile([128, 512], BF16, tag="lnb", name="rstd")
        with nc.allow_low_precision(reason="bf16 rstd is within LN tolerance"):
            nc.vector.reciprocal(out=rstd, in_=sd)
        bln = lnpool.tile([128, 512], BF16, tag="lnb", name="bln")
        nc.vector.scalar_tensor_tensor(out=bln, in0=mu, scalar=-1.0,
                                       in1=rstd, op0=ALU.mult, op1=ALU.mult)
        lns.append((rstd, bln))
    st["lns"] = lns


def stage_C3(nc, p, st, pst):
    """Apply LN affine + gelu. Emitted AFTER the next body's W1 so this
    ACT/DVE backlog never delays W1's PSUM drains."""
    v = pviews(pst)
    ltpool, gpool = p["lt"], p["one"]
    hs = st["hs"]
    gls = []
    for cch in range(2):
        rstd, bln = st["lns"][cch]
        gl = gpool.tile([128, HC, 512], BF16, tag="g", name=f"gl{cch}",
                        bufs=2)
        for m in range(HC):
            o1 = ltpool.tile([128, 512], BF16, tag="lt", name="o1")
            nc.vector.tensor_mul(out=o1, in0=hs[cch][:, m, :], in1=rstd)
            o2 = ltpool.tile([128, 512], BF16, tag="lt", name="o2")
            nc.vector.tensor_add(out=o2, in0=o1, in1=bln)
            nc.scalar.activation(out=gl[:, m, :], in_=o2, func=AF.Gelu,
                                 bias=v["be_v"][:, m:m + 1],
                                 scale=v["ga_v"][:, m:m + 1])
        gls.append(gl)
    st["gls"] = gls


def stage_D1(nc, p, st, pst):
    """Windowed-attention context via banded matmuls; ctxT copies rotate
    over DVE/ACT/Pool to keep the 3-bank PSUM rotation drained."""
    ppool, ps_sm = p["persist"], p["ps_sm"]
    U = st["U"]
    ctxT = ppool.tile([128, HC, TOK], BF16, tag="ctxT", name="ctxT", bufs=1)
    st["ctxT"] = ctxT
    k = 0
    for half in range(2):
        for hc in range(HC):
            pc = ps_sm.tile([128, 512], F32, tag="sm", name="pc")
            banded(nc, pc, half,
                   lambda j: st["x_sb"][:, j, hc * 128:(hc + 1) * 128],
                   lambda j: U[j])
            dst = ctxT[:, hc, 512 * half:512 * half + 512]
            if k % 2:
                nc.scalar.copy(out=dst, in_=pc)
            else:
                nc.vector.tensor_copy(out=dst, in_=pc)
            k += 1


def stage_D2(nc, p, st, pst):
    """W1 for both 512-chunks with shared stationaries; h copies + hsq."""
    v = pviews(pst)
    hpool, sqpool, ps_mm = p["one"], p["one"], p["ps_mm"]
    hs, qs = [], []
    for cch in range(2):
        hs.append(hpool.tile([128, HC, 512], BF16, tag="h", name=f"h{cch}",
                             bufs=4))
        qs.append(sqpool.tile([128, HC, 512], BF16, tag="hsq",
                              name=f"q{cch}", bufs=2))
    for m in range(HC):
        ph0 = ps_mm.tile([128, 512], F32, tag="mm", name="ph0")
        ph1 = ps_mm.tile([128, 512], F32, tag="mm", name="ph1")
        for k in range(KC):
            for cch, ph in ((0, ph0), (1, ph1)):
                c0 = 512 * cch
                rhs = (st["xT"][:, k, M + c0:M + c0 + 512] if k < HC
                       else st["ctxT"][:, k - HC, c0:c0 + 512])
                nc.tensor.matmul(ph, v["w1_v"][:, k, m * 128:(m + 1) * 128],
                                 rhs, start=(k == 0), stop=(k == KC - 1))
        for cch, ph in ((0, ph0), (1, ph1)):
            if cch:
                nc.scalar.activation(out=hs[cch][:, m, :], in_=ph,
                                     func=AF.Identity,
                                     bias=v["b1_v"][:, m:m + 1])
            else:
                nc.vector.tensor_scalar_add(out=hs[cch][:, m, :], in0=ph,
                                            scalar1=v["b1_v"][:, m:m + 1])
            nc.vector.tensor_mul(out=qs[cch][:, m, :], in0=hs[cch][:, m, :],
                                 in1=hs[cch][:, m, :])
    st["hs"], st["qs"] = hs, qs


def stage_E(nc, p, st, pst):
    """W2 projection, 0.5*base + 0.5*ctx combine, transpose, store."""
    v = pviews(pst)
    ppool, ltpool = p["persist"], p["lt"]
    ps_mm, ps_st = p["ps_mm"], p["ps_st"]
    logitsT = ppool.tile([L, TOK], F32, tag="logitsT", name="logitsT",
                         bufs=1)
    for cch in range(2):
        c0 = 512 * cch
        pl = ps_mm.tile([128, 512], F32, tag="mm", name="pl")
        for k in range(HC):
            nc.tensor.matmul(pl[:L, :], v["wst_v"][:, k, :],
                             st["gls"][cch][:, k, :],
                             start=(k == 0), stop=(k == HC - 1))
        blh = ltpool.tile([128, 512], F32, tag="blh", name="blh", bufs=2)
        nc.scalar.activation(out=blh[:L, :],
                             in_=st["sb10"][0:L, M + c0:M + c0 + 512],
                             func=AF.Identity, bias=v["bias9"], scale=0.5)
        nc.vector.scalar_tensor_tensor(out=logitsT[:, c0:c0 + 512],
                                       in0=pl[:L, :], scalar=0.5,
                                       in1=blh[:L, :],
                                       op0=ALU.mult, op1=ALU.add)
        po = ps_st.tile([128, 512], F32, tag="st", name="po")
        out_nat = ppool.tile([128, 4, L], F32, tag=f"onat{cch}",
                             name=f"onat{cch}")
        for j in range(4):
            jj = 4 * cch + j
            nc.tensor.transpose(po[:, j * L:(j + 1) * L],
                                logitsT[:, 128 * jj:128 * (jj + 1)],
                                v["id9"])
        if cch:
            nc.scalar.copy(out=out_nat,
                           in_=po[:, :4 * L].rearrange("p (j l) -> p j l",
                                                       l=L))
        else:
            nc.vector.tensor_copy(out=out_nat,
                                  in_=po[:, :4 * L].rearrange(
                                      "p (j l) -> p j l", l=L))
        nc.sync.dma_start(out=st["out_view"][:, 4 * cch:4 * cch + 4, :],
                          in_=out_nat)


def emit_steps(nc, p, io, n):
    """Emit n pipeline steps + drain; self-contained (fill from scratch)."""
    sts = [None] * n
    for i in range(n):
        sts[i] = stage_Aload(nc, p, io)
        cur = sts[i - 1] if i >= 1 else None   # freshest fully-loaded params
        if i >= 1:
            stage_B(nc, p, sts[i - 1], sts[i - 1])
        if i >= 2:
            stage_C1(nc, p, sts[i - 2], cur)
            stage_C2a(nc, p, sts[i - 2], cur)
        if i >= 1:
            stage_D1(nc, p, sts[i - 1], sts[i - 1])
        stage_Acomp(nc, p, sts[i])
        if i >= 2:
            stage_C2(nc, p, sts[i - 2], cur)
        if i >= 1:
            stage_D2(nc, p, sts[i - 1], sts[i - 1])
        if i >= 2:
            stage_C3(nc, p, sts[i - 2], cur)
            stage_E(nc, p, sts[i - 2], cur)
    # drain
    last = sts[n - 1]
    stage_B(nc, p, last, last)
    if n >= 2:
        stage_C1(nc, p, sts[n - 2], last)
        stage_C2a(nc, p, sts[n - 2], last)
    stage_D1(nc, p, last, last)
    if n >= 2:
        stage_C2(nc, p, sts[n - 2], last)
    stage_D2(nc, p, last, last)
    if n >= 2:
        stage_C3(nc, p, sts[n - 2], last)
        stage_E(nc, p, sts[n - 2], last)
    stage_C1(nc, p, last, last)
    stage_C2a(nc, p, last, last)
    stage_C2(nc, p, last, last)
    stage_C3(nc, p, last, last)
    stage_E(nc, p, last, last)


def build(rep=1, unroll=None):
    nc = bacc.Bacc("TRN2", target_bir_lowering=False, debug=False,
                   num_devices=8)

    xt_d = nc.dram_tensor("xt_loc", [128, HC * NJ * 128], BF16,
                          kind="ExternalInput").ap()
    x_d = nc.dram_tensor("x_loc", [NJ * 128, H], BF16,
                         kind="ExternalInput").ap()
    pb_d = nc.dram_tensor("pblob", [128, PB2], BF16,
                          kind="ExternalInput").ap()
    pf_d = nc.dram_tensor("pfblob", [128, PF], F32,
                          kind="ExternalInput").ap()
    out_d = nc.dram_tensor("out_loc", [TOK, L], F32,
                           kind="ExternalOutput").ap()

    io = (xt_d, x_d, pb_d, pf_d, out_d)

    with tile.TileContext(nc) as tc, ExitStack() as ctx:
        p = make_pools(tc, ctx)
        if rep == 1:
            emit_steps(nc, p, io, 1)
        else:
            if unroll is None:
                unroll = next(u for u in (UNROLL, 64, 32, 16, 8, 4, 2, 1)
                              if rep % u == 0)
            with tc.For_i(0, rep // unroll):
                emit_steps(nc, p, io, unroll)
    nc.compile()
    return nc


def make_host_inputs(sequence_output, Wc, bc, wa, ba, W1, b1, gamma, beta,
                     W2, b2):
    x = np.asarray(sequence_output, np.float32)
    bf = ml_dtypes.bfloat16

    pb = np.zeros((128, PB2), dtype=bf)
    w1 = np.asarray(W1, np.float32)
    pb[:, W1C:W1C + KC * H] = (
        w1.reshape(KC, 128, H).transpose(1, 0, 2).reshape(128, KC * H))
    pb[:, WSTC:WSTC + HC * L] = (
        np.asarray(W2, np.float32).reshape(HC, 128, L)
        .transpose(1, 0, 2).reshape(128, HC * L))
    wac = np.zeros((128, HC, 33), np.float32)
    wac[:, :, :L] = (np.asarray(Wc, np.float32)
                     .reshape(HC, 128, L).transpose(1, 0, 2))
    wac[:, :, 32] = np.asarray(wa, np.float32).reshape(HC, 128).T
    pb[:, WAC:WAC + HC * 33] = wac.reshape(128, HC * 33)
    pb[:, ONEC:ONEC + 128] = 1.0
    r_idx = np.arange(128)[:, None]
    u_idx = np.arange(UW)[None, :]
    pb[:, MSKC:MSKC + UW] = ((u_idx - r_idx >= 0) &
                             (u_idx - r_idx <= 2 * M)).astype(np.float32)

    pf_shared = np.zeros((128, PF), np.float32)
    pf_shared[:, B1C:B1C + HC] = np.asarray(b1, np.float32).reshape(HC, 128).T
    pf_shared[:, GAC:GAC + HC] = np.asarray(gamma, np.float32).reshape(HC, 128).T
    pf_shared[:, BEC:BEC + HC] = np.asarray(beta, np.float32).reshape(HC, 128).T
    pf_shared[:L, B9C] = np.asarray(bc, np.float32) + np.asarray(b2, np.float32)
    pf_shared[0, E0C] = 1.0
    pf_shared[32, E0C] = 1.0
    pf_shared[:L, ID9C:ID9C + L] = np.eye(L, dtype=np.float32)
    pf_shared[:, EPSC] = EPS
    # ba: softmax is shift-invariant, and scores feed nothing else -> drop it.

    in_maps = []
    for c in range(8):
        b, s0 = c // 2, TOK * (c % 2)
        x_loc = np.zeros((NJ * 128, H), dtype=bf)
        lo, hi = max(0, s0 - M), min(S, s0 + TOK + M)
        dst = lo - (s0 - M)
        x_loc[dst:dst + hi - lo] = x[b, lo:hi].astype(bf)
        # xT[p, hc, flat] = x_loc[flat, hc*128+p], flattened to [128, HC*1152]
        xt_loc = np.ascontiguousarray(
            x_loc.reshape(NJ * 128, HC, 128).transpose(2, 1, 0)
            .reshape(128, HC * NJ * 128))
        f = np.arange(128)[:, None] + 128 * np.arange(NJ)[None, :]
        g = s0 + f - M
        emask_np = ((g >= 0) & (g < S) & (f < FLAT)).astype(np.float32)
        pf_c = pf_shared.copy()
        pf_c[:, EMC:EMC + NJ] = emask_np
        in_maps.append({"xt_loc": xt_loc, "x_loc": x_loc, "pblob": pb,
                        "pfblob": pf_c})
    return in_maps


_cache = {}


def kernel(**inputs):
    if "nc" not in _cache:
        _cache["nc"] = build(rep=1)
    nc = _cache["nc"]
    in_maps = make_host_inputs(**inputs)
    res = run_bass_kernel_spmd(nc, in_maps, core_ids=list(range(8)))
    out = np.zeros((B, S, L), np.float32)
    for c in range(8):
        b, s0 = c // 2, TOK * (c % 2)
        out[b, s0:s0 + TOK] = res.results[c]["out_loc"]
    return out


# revision 26
# speedup vs baseline: 1.1695x; 1.0024x over previous
"""ContextAwareSpanClassifier Trainium2 Bass kernel (bf16, software-pipelined).

Problem (hardcoded): B=4, S=2048, H=768, L=9, M=5 (window W=11).
  base_logits = x @ Wc + bc
  s = x . wa + ba ; windowed softmax over [t-5, t+5] (seq-edge masked)
  ctx[t] = sum_o attn[t,o] * x[t+o]
  h = gelu_erf(LN(cat(x,ctx) @ W1 + b1) * gamma + beta)
  out = 0.5*base_logits + 0.5*(h @ W2 + b2)

Sharding: data parallel over B*S = 8192 tokens -> 8 cores x 1024 tokens
(core c: batch c//2, seq half (c%2)*1024) with 5-token zero-padded halos.
Params replicated. ba shift cancels in softmax and is otherwise unused.

Three-deep software pipeline: each unrolled step emits stages for three
bodies so the PE queue never waits on a cross-engine chain:
  step(i): Aload_i   loads for body i (xT XBAR transpose, params, x_sb)
           B_{i-1}   softmax denominator banded (PE) -> recip -> A-muls
           C1_{i-2}  LN stats matmuls (PE; covers B's recip/A DVE latency)
           D1_{i-1}  ctx banded (PE) + ctxT copies (DVE/ACT/Pool rotate)
           Acomp_i   scores sweep (PE; xT DMA has had a step of lead),
                     pt_sc transposes, softmax exp -> e_col -> U tiles
           C2_{i-2}  LN scalars (ln+exp rstd: shares the ACT exp table
                     with softmax -> only 2 table loads/step), apply, gelu
           D2_{i-1}  W1 both 512-chunks interleaved (each stationary
                     loaded once), h copies + hsq
           E_{i-2}   W2 proj, base+ctx combine, out transposes, store
PE stream/step: denom, stats, ctx, scores, W1, W2 -- every cross-engine
wait is covered by PE work from a neighboring body. 2 ACT table loads
per step (exp-set: softmax exp + LN ln/exp-rstd; gelu-set: gelu, with
Identity/Copy ops valid in any set). B/C/E read params from the newest
loaded generation (values identical) so pb/pf need only 2 buffers.
"""

from contextlib import ExitStack

import numpy as np
import ml_dtypes

import concourse.bass as bass
import concourse.tile as tile
from concourse import bacc, mybir
from concourse.bass_utils import run_bass_kernel_spmd

F32 = mybir.dt.float32
BF16 = mybir.dt.bfloat16
AF = mybir.ActivationFunctionType
ALU = mybir.AluOpType

B, S, H = 4, 2048, 768
L, M = 9, 5
TOK = 1024             # tokens per core
NT = 8                 # 128-token output tiles per core
NJ = 9                 # x storage tiles (tile 8 has 10 valid rows)
FLAT = TOK + 2 * M     # 1034
FPAD = 1040
HC = H // 128          # 6
KC = 2 * H // 128      # 12
WB = 2 * M + 1         # 11
UW = 138               # skew-band width: 128 + 10
EPS = 1e-5
UNROLL = 128

# bf16 param blob column offsets (small constants first so the leading DMA
# slice unblocks transposes/scores while the big W1 slices stream in)
WAC = 0                       # [128, HC*33] per-k [Wc | pad | wa@32] stationaries
ONEC = WAC + HC * 33          # [128, 128] ones
MSKC = ONEC + 128             # [128, UW] skew-band mask
W1C = MSKC + UW               # [128, KC*H] w1[p, k*H + m] = W1[k*128+p, m]
WSTC = W1C + KC * H           # [128, HC*L] W2 k-slabs (gl part only)
PB2 = WSTC + HC * L
SMALL = W1C                   # leading small-constant slice width

# f32 param blob column offsets
EMC = 0                       # [128, NJ] edge mask (per-core)
B1C = EMC + NJ                # [128, HC]
GAC = B1C + HC                # [128, HC]
BEC = GAC + HC                # [128, HC]
B9C = BEC + HC                # [:9] bias9 = bc + b2
E0C = B9C + 1                 # [128,1] e0 basis column
ID9C = E0C + 1                # [:9, 9] eye(9)
EPSC = ID9C + L               # [128,1] eps
PF = EPSC + 1

INV_H = 1.0 / H


def make_pools(tc, ctx):
    p = {}
    p["const"] = ctx.enter_context(tc.tile_pool(name="const", bufs=2))
    p["persist"] = ctx.enter_context(tc.tile_pool(name="persist", bufs=2))
    p["one"] = ctx.enter_context(tc.tile_pool(name="one", bufs=1))
    p["small"] = ctx.enter_context(tc.tile_pool(name="small", bufs=2))
    p["ln"] = ctx.enter_context(tc.tile_pool(name="ln", bufs=4))
    p["lt"] = ctx.enter_context(tc.tile_pool(name="lt", bufs=3))
    p["ps_mm"] = ctx.enter_context(tc.tile_pool(name="ps_mm", bufs=3, space="PSUM"))
    p["ps_sm"] = ctx.enter_context(tc.tile_pool(name="ps_sm", bufs=3, space="PSUM"))
    p["ps_st"] = ctx.enter_context(tc.tile_pool(name="ps_st", bufs=2, space="PSUM"))
    return p


def pviews(st):
    pb, pf = st["pb"], st["pf"]
    return {
        "w1_v": pb[:, W1C:W1C + KC * H].rearrange("p (k m) -> p k m", k=KC),
        "wst_v": pb[:, WSTC:WSTC + HC * L].rearrange("p (k l) -> p k l", k=HC),
        "wa_v": pb[:, WAC:WAC + HC * 33].rearrange("p (k c) -> p k c", k=HC),
        "ones_v": pb[:, ONEC:ONEC + 128],
        "msk_v": pb[:, MSKC:MSKC + UW],
        "emask": pf[:, EMC:EMC + NJ],
        "b1_v": pf[:, B1C:B1C + HC],
        "ga_v": pf[:, GAC:GAC + HC],
        "be_v": pf[:, BEC:BEC + HC],
        "bias9": pf[:L, B9C:B9C + 1],
        "e0": pf[:, E0C:E0C + 1],
        "id9": pf[:L, ID9C:ID9C + L],
        "eps_v": pf[:, EPSC:EPSC + 1],
    }


def stage_Aload(nc, p, io):
    """Issue all DMA loads for a new body; returns its state dict.

    xT arrives pre-transposed from the host (plain DMA, no XBAR); big
    transfers are spread across SP/ACT/DVE HWDGE queues so multiple DMA
    rings run in parallel."""
    (xt_d, x_d, pb_d, pf_d, out_d) = io
    ppool, cpool = p["persist"], p["const"]
    st = {}
    st["xT"] = ppool.tile([128, HC, NJ * 128], BF16, tag="xT", name="xT")
    xt_view = st["xT"].rearrange("p h f -> p (h f)")
    half = HC * NJ * 128 // 2
    nc.sync.dma_start(out=xt_view[:, :half], in_=xt_d[:, :half])
    nc.sync.dma_start(out=xt_view[:, half:], in_=xt_d[:, half:])
    st["pb"] = cpool.tile([128, PB2], BF16, tag="pb", name="pb")
    nc.scalar.dma_start(out=st["pb"][:, :SMALL], in_=pb_d[:, :SMALL])
    st["pf"] = cpool.tile([128, PF], F32, tag="pf", name="pf")
    nc.scalar.dma_start(out=st["pf"], in_=pf_d)
    st["x_sb"] = ppool.tile([128, NJ, H], BF16, tag="x", name="x_sb")
    x_view = x_d.rearrange("(j p) h -> p j h", p=128)
    for a, b in ((0, 5), (5, 9)):
        nc.gpsimd.dma_start(out=st["x_sb"][:, a:b, :], in_=x_view[:, a:b, :])
    for (a, b), eng in (((W1C, W1C + 4 * H), nc.scalar),
                        ((W1C + 4 * H, W1C + 8 * H), nc.gpsimd),
                        ((W1C + 8 * H, PB2), nc.sync)):
        eng.dma_start(out=st["pb"][:, a:b], in_=pb_d[:, a:b])
    st["out_view"] = out_d.rearrange("(j p) l -> p j l", p=128)
    return st


def stage_Acomp(nc, p, st):
    """Scores/base sweep + pt_sc transposes + softmax exp/e_col/U tiles."""
    v = pviews(st)
    ppool, spool = p["persist"], p["small"]
    ps_mm, ps_sm = p["ps_mm"], p["ps_sm"]

    sb10 = ppool.tile([33, FPAD], F32, tag="sb10", name="sb10", bufs=3)
    st["sb10"] = sb10
    pt_sc = ps_sm.tile([128, 512], F32, tag="sm", name="pt_sc")
    nc.vector.memset(pt_sc[:, :16], 0.0)

    for ci in range(3):
        c0, n = ((0, 512), (512, 512), (1024, 10))[ci]
        ps = ps_mm.tile([128, 512], F32, tag="mm", name="sc_ps")
        for hc in range(HC):
            nc.tensor.matmul(ps[:33, :n], v["wa_v"][:, hc, :],
                             st["xT"][:, hc, c0:c0 + n],
                             start=(hc == 0), stop=(hc == HC - 1))
        if ci % 2:
            nc.vector.tensor_copy(out=sb10[:, c0:c0 + n], in_=ps[:33, :n])
        else:
            nc.scalar.copy(out=sb10[:, c0:c0 + n], in_=ps[:33, :n])
    for j in range(NJ):
        n = 128 if j < NJ - 1 else 10
        nc.tensor.transpose(pt_sc[:n, j:j + 1],
                            sb10[32:33, 128 * j:128 * j + n],
                            v["e0"][32:33, :])

    e_tmp = spool.tile([128, NJ], F32, tag="etmp", name="e_tmp")
    nc.scalar.activation(out=e_tmp, in_=pt_sc[:, :NJ], func=AF.Exp)
    e_col = spool.tile([128, NJ], F32, tag="ecol", name="e_col")
    nc.vector.tensor_mul(out=e_col, in0=e_tmp, in1=v["emask"])
    U = [None] * NJ
    for j in range(NJ):
        u = spool.tile([128, UW], BF16, tag=f"u{j}", name=f"u{j}", bufs=2)
        nc.gpsimd.tensor_scalar_mul(out=u, in0=v["msk_v"],
                                    scalar1=e_col[:, j:j + 1])
        U[j] = u
    st["U"] = U


def banded(nc, out_bank, half, lhs_of_j, rhs_of_j):
    # out_bank covers dst cols [512*half, 512*half+512); src tile j
    # contributes dst cols [128j-10, 128j+128), 10-col overlaps accumulate
    # in PSUM (start only on the bank's first writer)
    base = 512 * half
    first = True
    for j in range(4 * half, 4 * half + 5):
        lo = max(128 * j - 10, 0)
        hi = min(128 * j + 128, TOK)
        lo, hi = max(lo, base), min(hi, base + 512)
        if lo >= hi:
            continue
        ua = lo - (128 * j - 10)
        nc.tensor.matmul(out_bank[:, lo - base:hi - base],
                         lhs_of_j(j), rhs_of_j(j)[:, ua:ua + hi - lo],
                         start=first, stop=(j == 4 * half + 4))
        first = False


def stage_B(nc, p, st, pst):
    """Softmax denominator banded (PE) -> reciprocal -> A-muls (in U)."""
    v = pviews(pst)
    ppool, ps_sm = p["persist"], p["ps_sm"]
    U = st["U"]
    r_rep = ppool.tile([128, TOK], BF16, tag="rrep", name="r_rep")
    st["r_rep"] = r_rep
    pds = []
    for half in range(2):
        pd = ps_sm.tile([128, 512], F32, tag="sm", name="pd")
        banded(nc, pd, half, lambda j: v["ones_v"], lambda j: U[j])
        pds.append(pd)
    with nc.allow_low_precision(reason="bf16 softmax denom recip"):
        for half in range(2):
            nc.vector.reciprocal(out=r_rep[:, 512 * half:512 * half + 512],
                                 in_=pds[half])
    # A_j = U_j * r built in place (disjoint column pieces per half)
    for half in range(2):
        base = 512 * half
        for j in range(4 * half, 4 * half + 5):
            glo = max(128 * j - 10, base)
            ghi = min(128 * j + 128, base + 512)
            if glo >= ghi:
                continue
            ua = glo - (128 * j - 10)
            nc.vector.tensor_mul(out=U[j][:, ua:ua + ghi - glo],
                                 in0=U[j][:, ua:ua + ghi - glo],
                                 in1=r_rep[:, glo:ghi])


def stage_C1(nc, p, st, pst):
    """LN stats matmuls on PE (sum h, sum h^2 per 512-chunk)."""
    v = pviews(pst)
    ps_st = p["ps_st"]
    hs, qs = st["hs"], st["qs"]
    st["ps_stats"] = []
    for cch in range(2):
        ps_s = ps_st.tile([128, 512], F32, tag="st", name="ps_s")
        for m in range(HC):
            nc.tensor.matmul(ps_s, v["ones_v"], hs[cch][:, m, :],
                             start=(m == 0), stop=(m == HC - 1))
        ps_q = ps_st.tile([128, 512], F32, tag="st", name="ps_q")
        for m in range(HC):
            nc.tensor.matmul(ps_q, v["ones_v"], qs[cch][:, m, :],
                             start=(m == 0), stop=(m == HC - 1))
        st["ps_stats"].append((ps_s, ps_q))


def stage_C2a(nc, p, st, pst):
    """Drain the four stats PSUM banks immediately (mu via ACT, var via
    DVE) so the bank rotation never waits on later-queued engine work."""
    lnpool = p["ln"]
    st["lnmv"] = []
    for cch in range(2):
        ps_s, ps_q = st["ps_stats"][cch]
        mu = lnpool.tile([128, 512], F32, tag="lnmu", name="mu", bufs=2)
        nc.scalar.activation(out=mu, in_=ps_s, func=AF.Copy, scale=INV_H)
        musq = lnpool.tile([128, 512], F32, tag="ln", name="musq")
        nc.vector.tensor_mul(out=musq, in0=mu, in1=mu)
        var = lnpool.tile([128, 512], F32, tag="ln", name="var")
        nc.vector.scalar_tensor_tensor(out=var, in0=ps_q, scalar=INV_H,
                                       in1=musq, op0=ALU.mult,
                                       op1=ALU.subtract)
        st["lnmv"].append((mu, var))


def stage_C2(nc, p, st, pst):
    """LN scalars: sqrt+recip rstd, bln (cheap; PSUM-adjacent)."""
    v = pviews(pst)
    lnpool = p["ln"]
    lns = []
    for cch in range(2):
        mu, var = st["lnmv"][cch]
        sd = lnpool.tile([128, 512], F32, tag="ln", name="sd")
        nc.scalar.activation(out=sd, in_=var, func=AF.Sqrt, bias=v["eps_v"])
        rstd = lnpool.tile([128, 512], BF16, tag="lnb", name="rstd")
        with nc.allow_low_precision(reason="bf16 rstd is within LN tolerance"):
            nc.vector.reciprocal(out=rstd, in_=sd)
        bln = lnpool.tile([128, 512], BF16, tag="lnb", name="bln")
        nc.vector.scalar_tensor_tensor(out=bln, in0=mu, scalar=-1.0,
                                       in1=rstd, op0=ALU.mult, op1=ALU.mult)
        lns.append((rstd, bln))
    st["lns"] = lns


def stage_C3(nc, p, st, pst):
    """Apply LN affine + gelu. Emitted AFTER the next body's W1 so this
    ACT/DVE backlog never delays W1's PSUM drains."""
    v = pviews(pst)
    ltpool, gpool = p["lt"], p["one"]
    hs = st["hs"]
    gls = []
    for cch in range(2):
        rstd, bln = st["lns"][cch]
        gl = gpool.tile([128, HC, 512], BF16, tag="g", name=f"gl{cch}",
                        bufs=2)
        for m in range(HC):
            o1 = ltpool.tile([128, 512], BF16, tag="lt", name="o1")
            nc.vector.tensor_mul(out=o1, in0=hs[cch][:, m, :], in1=rstd)
            o2 = ltpool.tile([128, 512], BF16, tag="lt", name="o2")
            nc.vector.tensor_add(out=o2, in0=o1, in1=bln)
            nc.scalar.activation(out=gl[:, m, :], in_=o2, func=AF.Gelu,
                                 bias=v["be_v"][:, m:m + 1],
                                 scale=v["ga_v"][:, m:m + 1])
        gls.append(gl)
    st["gls"] = gls


def stage_D1(nc, p, st, pst):
    """Windowed-attention context via banded matmuls; ctxT copies rotate
    over DVE/ACT/Pool to keep the 3-bank PSUM rotation drained."""
    ppool, ps_sm = p["persist"], p["ps_sm"]
    U = st["U"]
    ctxT = ppool.tile([128, HC, TOK], BF16, tag="ctxT", name="ctxT", bufs=1)
    st["ctxT"] = ctxT
    k = 0
    for half in range(2):
        for hc in range(HC):
            pc = ps_sm.tile([128, 512], F32, tag="sm", name="pc")
            banded(nc, pc, half,
                   lambda j: st["x_sb"][:, j, hc * 128:(hc + 1) * 128],
                   lambda j: U[j])
            dst = ctxT[:, hc, 512 * half:512 * half + 512]
            if k % 2:
                nc.scalar.copy(out=dst, in_=pc)
            else:
                nc.vector.tensor_copy(out=dst, in_=pc)
            k += 1


def stage_D2(nc, p, st, pst):
    """W1 for both 512-chunks with shared stationaries; h copies + hsq."""
    v = pviews(pst)
    hpool, sqpool, ps_mm = p["one"], p["one"], p["ps_mm"]
    hs, qs = [], []
    for cch in range(2):
        hs.append(hpool.tile([128, HC, 512], BF16, tag="h", name=f"h{cch}",
                             bufs=4))
        qs.append(sqpool.tile([128, HC, 512], BF16, tag="hsq",
                              name=f"q{cch}", bufs=2))
    for m in range(HC):
        ph0 = ps_mm.tile([128, 512], F32, tag="mm", name="ph0")
        ph1 = ps_mm.tile([128, 512], F32, tag="mm", name="ph1")
        for k in range(KC):
            for cch, ph in ((0, ph0), (1, ph1)):
                c0 = 512 * cch
                rhs = (st["xT"][:, k, M + c0:M + c0 + 512] if k < HC
                       else st["ctxT"][:, k - HC, c0:c0 + 512])
                nc.tensor.matmul(ph, v["w1_v"][:, k, m * 128:(m + 1) * 128],
                                 rhs, start=(k == 0), stop=(k == KC - 1))
        for cch, ph in ((0, ph0), (1, ph1)):
            if cch:
                nc.scalar.activation(out=hs[cch][:, m, :], in_=ph,
                                     func=AF.Identity,
                                     bias=v["b1_v"][:, m:m + 1])
            else:
                nc.vector.tensor_scalar_add(out=hs[cch][:, m, :], in0=ph,
                                            scalar1=v["b1_v"][:, m:m + 1])
            nc.vector.tensor_mul(out=qs[cch][:, m, :], in0=hs[cch][:, m, :],
                                 in1=hs[cch][:, m, :])
    st["hs"], st["qs"] = hs, qs


def stage_E(nc, p, st, pst):
    """W2 projection, 0.5*base + 0.5*ctx combine, transpose, store."""
    v = pviews(pst)
    ppool, ltpool = p["persist"], p["lt"]
    ps_mm, ps_st = p["ps_mm"], p["ps_st"]
    logitsT = ppool.tile([L, TOK], F32, tag="logitsT", name="logitsT",
                         bufs=1)
    for cch in range(2):
        c0 = 512 * cch
        pl = ps_mm.tile([128, 512], F32, tag="mm", name="pl")
        for k in range(HC):
            nc.tensor.matmul(pl[:L, :], v["wst_v"][:, k, :],
                             st["gls"][cch][:, k, :],
                             start=(k == 0), stop=(k == HC - 1))
        blh = ltpool.tile([128, 512], F32, tag="blh", name="blh", bufs=2)
        nc.scalar.activation(out=blh[:L, :],
                             in_=st["sb10"][0:L, M + c0:M + c0 + 512],
                             func=AF.Identity, bias=v["bias9"], scale=0.5)
        nc.vector.scalar_tensor_tensor(out=logitsT[:, c0:c0 + 512],
                                       in0=pl[:L, :], scalar=0.5,
                                       in1=blh[:L, :],
                                       op0=ALU.mult, op1=ALU.add)
        po = ps_st.tile([128, 512], F32, tag="st", name="po")
        out_nat = ppool.tile([128, 4, L], F32, tag=f"onat{cch}",
                             name=f"onat{cch}")
        for j in range(4):
            jj = 4 * cch + j
            nc.tensor.transpose(po[:, j * L:(j + 1) * L],
                                logitsT[:, 128 * jj:128 * (jj + 1)],
                                v["id9"])
        if cch:
            nc.scalar.copy(out=out_nat,
                           in_=po[:, :4 * L].rearrange("p (j l) -> p j l",
                                                       l=L))
        else:
            nc.vector.tensor_copy(out=out_nat,
                                  in_=po[:, :4 * L].rearrange(
                                      "p (j l) -> p j l", l=L))
        nc.sync.dma_start(out=st["out_view"][:, 4 * cch:4 * cch + 4, :],
                          in_=out_nat)


def emit_steps(nc, p, io, n):
    """Emit n pipeline steps + drain; self-contained (fill from scratch)."""
    sts = [None] * n
    for i in range(n):
        sts[i] = stage_Aload(nc, p, io)
        cur = sts[i - 1] if i >= 1 else None   # freshest fully-loaded params
        if i >= 1:
            stage_B(nc, p, sts[i - 1], sts[i - 1])
        if i >= 2:
            stage_C1(nc, p, sts[i - 2], cur)
            stage_C2a(nc, p, sts[i - 2], cur)
        if i >= 1:
            stage_D1(nc, p, sts[i - 1], sts[i - 1])
        stage_Acomp(nc, p, sts[i])
        if i >= 2:
            stage_C2(nc, p, sts[i - 2], cur)
        if i >= 1:
            stage_D2(nc, p, sts[i - 1], sts[i - 1])
        if i >= 2:
            stage_C3(nc, p, sts[i - 2], cur)
            stage_E(nc, p, sts[i - 2], cur)
    # drain
    last = sts[n - 1]
    stage_B(nc, p, last, last)
    if n >= 2:
        stage_C1(nc, p, sts[n - 2], last)
        stage_C2a(nc, p, sts[n - 2], last)
    stage_D1(nc, p, last, last)
    if n >= 2:
        stage_C2(nc, p, sts[n - 2], last)
    stage_D2(nc, p, last, last)
    if n >= 2:
        stage_C3(nc, p, sts[n - 2], last)
        stage_E(nc, p, sts[n - 2], last)
    stage_C1(nc, p, last, last)
    stage_C2a(nc, p, last, last)
    stage_C2(nc, p, last, last)
    stage_C3(nc, p, last, last)
    stage_E(nc, p, last, last)


def build(rep=1, unroll=None):
    nc = bacc.Bacc("TRN2", target_bir_lowering=False, debug=False,
                   num_devices=8)

    xt_d = nc.dram_tensor("xt_loc", [128, HC * NJ * 128], BF16,
                          kind="ExternalInput").ap()
    x_d = nc.dram_tensor("x_loc", [NJ * 128, H], BF16,
                         kind="ExternalInput").ap()
    pb_d = nc.dram_tensor("pblob", [128, PB2], BF16,
                          kind="ExternalInput").ap()
    pf_d = nc.dram_tensor("pfblob", [128, PF], F32,
                          kind="ExternalInput").ap()
    out_d = nc.dram_tensor("out_loc", [TOK, L], F32,
                           kind="ExternalOutput").ap()

    io = (xt_d, x_d, pb_d, pf_d, out_d)

    with tile.TileContext(nc) as tc, ExitStack() as ctx:
        p = make_pools(tc, ctx)
        if rep == 1:
            emit_steps(nc, p, io, 1)
        else:
            if unroll is None:
                unroll = next(u for u in (UNROLL, 64, 32, 16, 8, 4, 2, 1)
                              if rep % u == 0)
            with tc.For_i(0, rep // unroll):
                emit_steps(nc, p, io, unroll)
    nc.compile()
    return nc


def make_host_inputs(sequence_output, Wc, bc, wa, ba, W1, b1, gamma, beta,
                     W2, b2):
    x = np.asarray(sequence_output, np.float32)
    bf = ml_dtypes.bfloat16

    pb = np.zeros((128, PB2), dtype=bf)
    w1 = np.asarray(W1, np.float32)
    pb[:, W1C:W1C + KC * H] = (
        w1.reshape(KC, 128, H).transpose(1, 0, 2).reshape(128, KC * H))
    pb[:, WSTC:WSTC + HC * L] = (
        np.asarray(W2, np.float32).reshape(HC, 128, L)
        .transpose(1, 0, 2).reshape(128, HC * L))
    wac = np.zeros((128, HC, 33), np.float32)
    wac[:, :, :L] = (np.asarray(Wc, np.float32)
                     .reshape(HC, 128, L).transpose(1, 0, 2))
    wac[:, :, 32] = np.asarray(wa, np.float32).reshape(HC, 128).T
    pb[:, WAC:WAC + HC * 33] = wac.reshape(128, HC * 33)
    pb[:, ONEC:ONEC + 128] = 1.0
    r_idx = np.arange(128)[:, None]
    u_idx = np.arange(UW)[None, :]
    pb[:, MSKC:MSKC + UW] = ((u_idx - r_idx >= 0) &
                             (u_idx - r_idx <= 2 * M)).astype(np.float32)

    pf_shared = np.zeros((128, PF), np.float32)
    pf_shared[:, B1C:B1C + HC] = np.asarray(b1, np.float32).reshape(HC, 128).T
    pf_shared[:, GAC:GAC + HC] = np.asarray(gamma, np.float32).reshape(HC, 128).T
    pf_shared[:, BEC:BEC + HC] = np.asarray(beta, np.float32).reshape(HC, 128).T
    pf_shared[:L, B9C] = np.asarray(bc, np.float32) + np.asarray(b2, np.float32)
    pf_shared[0, E0C] = 1.0
    pf_shared[32, E0C] = 1.0
    pf_shared[:L, ID9C:ID9C + L] = np.eye(L, dtype=np.float32)
    pf_shared[:, EPSC] = EPS
    # ba: softmax is shift-invariant, and scores feed nothing else -> drop it.

    in_maps = []
    for c in range(8):
        b, s0 = c // 2, TOK * (c % 2)
        x_loc = np.zeros((NJ * 128, H), dtype=bf)
        lo, hi = max(0, s0 - M), min(S, s0 + TOK + M)
        dst = lo - (s0 - M)
        x_loc[dst:dst + hi - lo] = x[b, lo:hi].astype(bf)
        # xT[p, hc, flat] = x_loc[flat, hc*128+p], flattened to [128, HC*1152]
        xt_loc = np.ascontiguousarray(
            x_loc.reshape(NJ * 128, HC, 128).transpose(2, 1, 0)
            .reshape(128, HC * NJ * 128))
        f = np.arange(128)[:, None] + 128 * np.arange(NJ)[None, :]
        g = s0 + f - M
        emask_np = ((g >= 0) & (g < S) & (f < FLAT)).astype(np.float32)
        pf_c = pf_shared.copy()
        pf_c[:, EMC:EMC + NJ] = emask_np
        in_maps.append({"xt_loc": xt_loc, "x_loc": x_loc, "pblob": pb,
                        "pfblob": pf_c})
    return in_maps


_cache = {}


def kernel(**inputs):
    if "nc" not in _cache:
        _cache["nc"] = build(rep=1)
    nc = _cache["nc"]
    in_maps = make_host_inputs(**inputs)
    res = run_bass_kernel_spmd(nc, in_maps, core_ids=list(range(8)))
    out = np.zeros((B, S, L), np.float32)
    for c in range(8):
        b, s0 = c // 2, TOK * (c % 2)
        out[b, s0:s0 + TOK] = res.results[c]["out_loc"]
    return out
